# revision 1
# baseline (speedup 1.0000x reference)
"""Trainium2 Bass kernel for nn_ClipForegroundEstimator.

Pipeline (per batch): two (1x1conv -> GroupNorm) blocks over [Fd,T] features,
a sigmoid classifier head, a text-prototype head over img_feats, and a
per-(batch, class) mean of the top-k values along T for both heads.

Sharding: data-parallel over batch. 8 cores x 2 batches each. All params
replicated. Each core returns a [128,2] score tile: col 0 = text head,
col 1 = cls head, with batch b0/b1 at partition offsets 0/32.

Top-k mean is computed without sorting: binary-search a per-series threshold t
with count(x > t) == k, then  topk_sum = k*t + sum(relu(x - t)).  The formula
is exact for any t in [x_(k+1), x_(k)], and membership mistakes within the
final search interval contribute only O(interval^2 * local_density) error,
so few iterations suffice. The count is split across DVE (is_gt+accum) and
ACT (Sign+accum) each iteration.

Profiled state (8x TRN2, NTFF traces): 320us/core, rel err 2.1e-5.
Breakdown: ~226us PE matmul stream (932 MMs @ ~243ns = fp32r N=512 issue
floor, weight loads hidden), ~49us HAM half-clock penalty (98us of MMs run
at K=4/8 because ~45us of DVE-sem waits -- GN stats chain + PSUM drain
handoffs -- re-throttle the PE ~20x), ~39us tail (9-iter cls search ~23us
+ ~13us fixed Tile drain barrier), ~13us DMA cold start.
Next lever, unattempted: keep the PE HAM-warm through the GN windows by
hoisting dependency-free matmuls (GN bias MMs, next batch's transposes)
into them via explicit ordering, or pipeline GN stats one superblock early.
Projected ~275-280us. Known erratum: bf16 tensor_tensor_reduce passes
CoreSim but crashes TRN2 hardware -- do not reintroduce.
"""

import numpy as np
import ml_dtypes

import concourse.bass as bass
import concourse.tile as tile
from concourse import bacc, mybir
from concourse.bass_utils import run_bass_kernel_spmd

f32 = mybir.dt.float32
f32r = mybir.dt.float32r
bf16 = mybir.dt.bfloat16
AL = mybir.AluOpType
AF = mybir.ActivationFunctionType
AX = mybir.AxisListType

# problem shapes (hardcoded per spec)
B, FD, T, O, TIMG, D, C = 16, 2048, 2048, 512, 2048, 512, 20
GROUPS, R_ACT, EPS = 32, 8, 1e-5
NCORES, BPC = 8, 2        # cores, batches per core
KT = FD // 128            # 16 k-tiles for layer1 contraction
MT = O // 128             # 4 m-tiles of output channels
DT = D // 128             # 4 k-tiles for D contraction
NSB = 4                   # T superblocks of 512
GN_N = (O // GROUPS) * T  # elements per group = 16*2048
N_ITERS = 13              # binary search iterations
DVE_COLS = 1152           # search count split: DVE [0:1152], ACT [1152:2048]
ACT_COLS = T - DVE_COLS

# partition rows of batches inside [128, T] logits tiles
ROW = (0, 32)


def _topk_search(nc, spool, scr, scrA, logits, kv, k2, ki, out_col,
                 fixed_unit_range=False, n_iters=N_ITERS):
    """Binary-search topk threshold for all series in `logits`, write
    mean-of-topk to out_col ([128,1] AP). kv/k2/ki: [128,1] APs with
    k, 2k-ACT_COLS, 1/k per partition. fixed_unit_range: values in [0,1]
    (sigmoid outputs) -> skip the min/max reduces."""
    sv = spool.tile([128, 8], f32, name="sv", tag="sv")
    mn, mx = sv[:, 0:1], sv[:, 1:2]
    mid, hw = sv[:, 2:3], sv[:, 3:4]
    cntD, tot, t1, nm = sv[:, 4:5], sv[:, 5:6], sv[:, 6:7], sv[:, 7:8]
    svb = spool.tile([128, 2], f32, name="svb", tag="svb")
    sa, sa2 = svb[:, 0:1], svb[:, 1:2]

    # state: hi (upper bound) and hw (halfwidth); probe mid = hi - hw.
    hi = mn  # reuse slot
    if fixed_unit_range:
        nc.vector.memset(hi, 1.0)
        nc.vector.memset(hw, 0.5)
        nc.vector.memset(mid, 0.5)
    else:
        nc.vector.tensor_reduce(mn, logits, AX.X, AL.min)
        nc.vector.tensor_reduce(mx, logits, AX.X, AL.max)
        nc.vector.tensor_tensor(hw, mx, mn, AL.subtract)
        nc.vector.tensor_scalar(hw, hw, 0.5, None, op0=AL.mult)
        nc.vector.tensor_copy(hi, mx)
        nc.vector.tensor_tensor(mid, mx, hw, AL.subtract)
    for _ in range(n_iters):
        # count #(x > mid): DVE is_gt+sum on [0:DVE_COLS];
        # ACT sign(mid - x)+sum on the rest (sa = #lt - #gt there)
        nc.scalar.activation(
            scrA, logits[:, DVE_COLS:], AF.Sign, bias=mid, scale=-1.0, accum_out=sa
        )
        nc.vector.tensor_scalar(
            scr, logits[:, :DVE_COLS], mid, None,
            op0=AL.is_gt, op1=AL.add, accum_out=cntD,
        )
        # tot = 2*cntD - sa ;  (tot >= 2k - ACT_COLS) <=> count >= k
        nc.vector.scalar_tensor_tensor(tot, cntD, 2.0, sa, op0=AL.mult, op1=AL.subtract)
        # ge -> hi' = mid + ge*hw ; hw /= 2 (off critical path) ; mid' = hi' - hw'
        nc.vector.tensor_scalar(t1, tot, k2, hw, op0=AL.is_ge, op1=AL.mult)
        nc.vector.tensor_scalar(hw, hw, 0.5, None, op0=AL.mult)
        nc.vector.tensor_tensor(hi, mid, t1, AL.add)
        nc.vector.tensor_tensor(mid, hi, hw, AL.subtract)
    lo = mid
    nc.vector.tensor_scalar(nm, lo, -1.0, None, op0=AL.mult)
    # sum(relu(x - lo)) on ACT in two chunks
    nc.scalar.activation(scr, logits[:, :DVE_COLS], AF.Relu, bias=nm, accum_out=sa)
    nc.scalar.activation(scrA, logits[:, DVE_COLS:], AF.Relu, bias=nm, accum_out=sa2)
    nc.vector.tensor_tensor(t1, sa, sa2, AL.add)
    nc.vector.scalar_tensor_tensor(tot, lo, kv, t1, op0=AL.mult, op1=AL.add)
    nc.vector.tensor_tensor(out_col, tot, ki, AL.mult)


def _body(tc, io):
    nc = tc.nc
    feat, img = io["feat"], io["img"]
    w1t, w2t, wct, tpt = io["w1t"], io["w2t"], io["wct"], io["tpt"]
    bias_pack, bc_pad = io["bias_pack"], io["bc_pad"]
    ind_i, ind_j, eye = io["ind_i"], io["ind_j"], io["eye"]
    kpack, scores = io["kpack"], io["scores"]

    import contextlib
    ctx = contextlib.ExitStack()
    with ctx:
        cpool = ctx.enter_context(tc.tile_pool(name="consts", bufs=1))
        fpool = ctx.enter_context(tc.tile_pool(name="fstream", bufs=8))
        ipool = ctx.enter_context(tc.tile_pool(name="imgstream", bufs=3))
        xpool = ctx.enter_context(tc.tile_pool(name="acts", bufs=1))
        spool = ctx.enter_context(tc.tile_pool(name="stats", bufs=2))
        scpool = ctx.enter_context(tc.tile_pool(name="scratch", bufs=2))
        wspool = ctx.enter_context(tc.tile_pool(name="wscaled", bufs=2))
        bigpool = ctx.enter_context(tc.tile_pool(name="bigs", bufs=1))
        psum = ctx.enter_context(tc.tile_pool(name="ps", bufs=8, space="PSUM"))

        # ---- persistent constants (large ones DMA'd lazily, see below) ----
        w1t_sb = cpool.tile([128, KT, O], f32r, name="w1t_sb")
        w2t_sb = cpool.tile([128, MT, O], bf16, name="w2t_sb")
        wct_sb = cpool.tile([128, DT, C], bf16, name="wct_sb")
        tpt_sb = cpool.tile([128, DT, C], f32r, name="tpt_sb")
        bp_sb = cpool.tile([128, 24], f32, name="bp_sb")
        nc.gpsimd.dma_start(out=bp_sb, in_=bias_pack)
        bc_sb = cpool.tile([128, 1], f32, name="bc_sb")
        nc.gpsimd.dma_start(out=bc_sb, in_=bc_pad)
        indi_sb = cpool.tile([128, MT, GROUPS], f32, name="indi_sb")
        nc.gpsimd.dma_start(out=indi_sb, in_=ind_i)
        indj_sb = cpool.tile([128, MT, 128], f32, name="indj_sb")
        nc.gpsimd.dma_start(out=indj_sb[:GROUPS], in_=ind_j)
        eye_sb = cpool.tile([128, 128], f32, name="eye_sb")
        nc.gpsimd.dma_start(out=eye_sb, in_=eye)
        eps_sb = cpool.tile([128, 1], f32, name="eps_sb")
        nc.vector.memset(eps_sb, EPS)
        kp_sb = cpool.tile([128, 6], f32, name="kp_sb")
        nc.gpsimd.dma_start(out=kp_sb, in_=kpack)

        textL = bigpool.tile([128, T], f32, name="textL")
        clsL = bigpool.tile([128, T], f32, name="clsL")
        nc.vector.memset(textL, 0.0)
        nc.vector.memset(clsL, 0.0)
        scr = bigpool.tile([128, DVE_COLS], bf16, name="scr")
        scrA = bigpool.tile([128, ACT_COLS], bf16, name="scrA")
        scout = bigpool.tile([128, 2], f32, name="scout")

        def bcol(base, m):
            return bp_sb[:, base + m : base + m + 1]

        for b in range(BPC):
            if b == 1:
                # ---- b1 text head FIRST: its search then overlaps layer1-b1
                imgT = bigpool.tile([128, DT, TIMG], f32r, name="imgT")
                _text_head(nc, psum, ipool, img, imgT, eye_sb, tpt_sb, textL, b)
                _topk_search(nc, spool, scr, scrA, textL,
                             kp_sb[:, 0:1], kp_sb[:, 1:2], kp_sb[:, 2:3],
                             scout[:, 0:1], n_iters=11)

            # ---------------- layer 1: X1 = W1 @ F + b1 ----------------------
            x1p = [xpool.tile([128, T], bf16, name=f"x1p{m}", tag=f"x1p{m}", bufs=2) for m in range(MT)]
            scq1 = spool.tile([128, MT, NSB, 2], f32, name="scq1")
            for nsb in range(NSB):
                ns0 = nsb * 512
                ps1 = [psum.tile([128, 512], f32, name=f"ps1{m}", tag="ps") for m in range(MT)]
                for k in range(KT):
                    if b == 0 and nsb == 0:
                        nc.sync.dma_start(out=w1t_sb[:, k, :], in_=w1t[k])
                    ft = fpool.tile([128, 512], f32r, name="ft")
                    nc.sync.dma_start(
                        out=ft, in_=feat[b, k * 128 : (k + 1) * 128, ns0 : ns0 + 512]
                    )
                    for m in range(MT):
                        nc.tensor.matmul(
                            ps1[m],
                            lhsT=w1t_sb[:, k, m * 128 : (m + 1) * 128],
                            rhs=ft,
                            start=(k == 0),
                            stop=(k == KT - 1),
                        )
                for m in range(MT):
                    xs = x1p[m][:, ns0 : ns0 + 512]
                    nc.vector.tensor_scalar(
                        xs, ps1[m], bcol(0, m), None,
                        op0=AL.add, op1=AL.add,
                        accum_out=scq1[:, m, nsb, 0:1],
                    )
                    sqs = scpool.tile([128, 512], bf16, name="sqs")
                    nc.scalar.activation(
                        sqs, ps1[m], AF.Square, bias=bcol(0, m),
                        accum_out=scq1[:, m, nsb, 1:2],
                    )

            if b == 0:
                # deferred medium consts: issued after layer1-b0's F stream
                for k in range(DT):
                    nc.gpsimd.dma_start(out=tpt_sb[:, k, :], in_=tpt[k])
                for k in range(MT):
                    nc.gpsimd.dma_start(out=w2t_sb[:, k, :], in_=w2t[k])
                for k in range(DT):
                    nc.gpsimd.dma_start(out=wct_sb[:, k, :], in_=wct[k])
                # ---- b0 text head after layer1 so F DMAs get priority
                imgT = bigpool.tile([128, DT, TIMG], f32r, name="imgT")
                _text_head(nc, psum, ipool, img, imgT, eye_sb, tpt_sb, textL, b)

            # ---- GN1 stats -> fold affine into W2 + bias2 -------------------
            pcb1p = _gn_stats(nc, psum, spool, indi_sb, indj_sb, eps_sb, scq1,
                              f"gn1b{b}")
            pcb1 = spool.tile([128, 2 * MT], f32, name="pcb1")
            nc.vector.tensor_copy(pcb1, pcb1p[:, 0 : 2 * MT])
            w2ts = [wspool.tile([128, O], bf16, name=f"w2ts{k}", tag=f"w2ts{k}", bufs=2)
                    for k in range(MT)]
            ngb1 = spool.tile([128, MT], bf16, name="ngb1")
            for k in range(MT):
                nc.vector.tensor_scalar(
                    w2ts[k], w2t_sb[:, k, :],
                    pcb1[:, 2 * k : 2 * k + 1], bcol(4, k),
                    op0=AL.mult, op1=AL.mult,
                )
                # negB = gamma*rm - beta
                nc.vector.tensor_scalar(
                    ngb1[:, k : k + 1], bcol(4, k),
                    pcb1[:, 2 * k + 1 : 2 * k + 2], bcol(8, k),
                    op0=AL.mult, op1=AL.subtract,
                )
            psb = psum.tile([128, 512], f32, name=f"psb{b}", tag="ps")
            for m in range(MT):
                for k in range(MT):
                    nc.tensor.matmul(
                        psb[:, m : m + 1],
                        lhsT=w2t_sb[:, k, m * 128 : (m + 1) * 128],
                        rhs=ngb1[:, k : k + 1],
                        start=(k == 0),
                        stop=(k == MT - 1),
                    )
            bias2 = spool.tile([128, MT], f32, name="bias2")
            for m in range(MT):
                nc.vector.tensor_tensor(
                    bias2[:, m : m + 1], bcol(12, m), psb[:, m : m + 1], AL.subtract
                )

            # ---------------- layer 2: X2 = W2n @ X1p + bias2 ----------------
            x2p = [xpool.tile([128, T], bf16, name=f"x2p{m}", tag=f"x2p{m}") for m in range(MT)]
            scq2 = spool.tile([128, MT, NSB, 2], f32, name="scq2")
            for m in range(MT):
                for nsb in range(NSB):
                    ns0 = nsb * 512
                    ps2 = psum.tile([128, 512], f32, name="ps2", tag="ps")
                    for k in range(MT):
                        nc.tensor.matmul(
                            ps2,
                            lhsT=w2ts[k][:, m * 128 : (m + 1) * 128],
                            rhs=x1p[k][:, ns0 : ns0 + 512],
                            start=(k == 0),
                            stop=(k == MT - 1),
                        )
                    xs2 = x2p[m][:, ns0 : ns0 + 512]
                    nc.vector.tensor_scalar(
                        xs2, ps2, bias2[:, m : m + 1], None,
                        op0=AL.add, op1=AL.add,
                        accum_out=scq2[:, m, nsb, 0:1],
                    )
                    sqs2 = scpool.tile([128, 512], bf16, name="sqs")
                    nc.scalar.activation(
                        sqs2, ps2, AF.Square, bias=bias2[:, m : m + 1],
                        accum_out=scq2[:, m, nsb, 1:2],
                    )

            # ---- GN2 stats -> fold affine into Wc + clsb --------------------
            pcb2p = _gn_stats(nc, psum, spool, indi_sb, indj_sb, eps_sb, scq2,
                              f"gn2b{b}")
            pcb2 = spool.tile([128, 2 * MT], f32, name="pcb2")
            nc.vector.tensor_copy(pcb2, pcb2p[:, 0 : 2 * MT])
            wcts = [wspool.tile([128, C], bf16, name=f"wcts{k}", tag=f"wcts{k}", bufs=2)
                    for k in range(MT)]
            ngb2 = spool.tile([128, MT], bf16, name="ngb2")
            for k in range(MT):
                nc.vector.tensor_scalar(
                    wcts[k], wct_sb[:, k, :],
                    pcb2[:, 2 * k : 2 * k + 1], bcol(16, k),
                    op0=AL.mult, op1=AL.mult,
                )
                nc.vector.tensor_scalar(
                    ngb2[:, k : k + 1], bcol(16, k),
                    pcb2[:, 2 * k + 1 : 2 * k + 2], bcol(20, k),
                    op0=AL.mult, op1=AL.subtract,
                )
            pscb = psum.tile([128, 512], f32, name=f"pscb{b}", tag="ps")
            for k in range(MT):
                nc.tensor.matmul(
                    pscb[:C, 0:1],
                    lhsT=wct_sb[:, k, :],
                    rhs=ngb2[:, k : k + 1],
                    start=(k == 0),
                    stop=(k == MT - 1),
                )
            clsb = spool.tile([128, 1], f32, name="clsb")
            nc.vector.tensor_tensor(clsb[:C], bc_sb[:C], pscb[:C, 0:1], AL.subtract)

            # ---------------- cls head: sigmoid(Wcn @ X2p + clsb) ------------
            r0 = ROW[b]
            for nq in range(4):
                psc = psum.tile([128, 512], f32, name="psc", tag="ps")
                for k in range(MT):
                    nc.tensor.matmul(
                        psc[:C],
                        lhsT=wcts[k],
                        rhs=x2p[k][:, nq * 512 : (nq + 1) * 512],
                        start=(k == 0),
                        stop=(k == MT - 1),
                    )
                nc.scalar.activation(
                    clsL[r0 : r0 + C, nq * 512 : (nq + 1) * 512],
                    psc[:C], AF.Sigmoid, bias=clsb[:C],
                )

        _topk_search(nc, spool, scr, scrA, clsL,
                     kp_sb[:, 3:4], kp_sb[:, 4:5], kp_sb[:, 5:6],
                     scout[:, 1:2], fixed_unit_range=True, n_iters=9)
        nc.sync.dma_start(out=scores.ap(), in_=scout)


def _gn_stats(nc, psum, spool, indi_sb, indj_sb, eps_sb, scq, lname):
    """GroupNorm statistics from per-channel (sum, sumsq) partials.

    ind_i is pre-scaled by 1/GN_N on the host, so the group matmul yields
    (mu, msq) partials directly. Returns a PSUM tile whose columns
    (2m, 2m+1) hold per-channel (rs, rs*mu) for m-tile m.
    """
    psg = psum.tile([128, 512], f32, name=f"psg_{lname}", tag="ps")
    for m in range(MT):
        nc.tensor.matmul(
            psg[:GROUPS, 0 : 2 * NSB],
            lhsT=indi_sb[:, m, :],
            rhs=scq[:, m].rearrange("p a b -> p (a b)"),
            start=(m == 0),
            stop=(m == MT - 1),
        )
    grp = spool.tile([128, 4], f32, name=f"grp_{lname}")
    # cols: 0=mu, 1=msq, 2=rs (after sqrt+recip), 3=rs*mu
    nc.vector.tensor_reduce(
        grp[:GROUPS, 0:2],
        psg[:GROUPS, 0 : 2 * NSB].rearrange("p (j s) -> p s j", j=NSB),
        AX.X, AL.add,
    )
    # -var = mu*mu - msq ; std = sqrt(-1*(-var) + eps)
    nc.vector.scalar_tensor_tensor(
        grp[:GROUPS, 2:3], grp[:GROUPS, 0:1], grp[:GROUPS, 0:1], grp[:GROUPS, 1:2],
        op0=AL.mult, op1=AL.subtract,
    )
    nc.scalar.activation(
        grp[:GROUPS, 2:3], grp[:GROUPS, 2:3], AF.Sqrt,
        bias=eps_sb[:GROUPS], scale=-1.0,
    )
    nc.vector.reciprocal(grp[:GROUPS, 2:3], grp[:GROUPS, 2:3])
    nc.vector.tensor_tensor(
        grp[:GROUPS, 3:4], grp[:GROUPS, 2:3], grp[:GROUPS, 0:1], AL.mult
    )
    pcb = psum.tile([128, 512], f32, name=f"pcb_{lname}", tag="ps")
    for m in range(MT):
        nc.tensor.matmul(
            pcb[:, 2 * m : 2 * m + 2],
            lhsT=indj_sb[:GROUPS, m, :],
            rhs=grp[:GROUPS, 2:4],
            start=True,
            stop=True,
        )
    return pcb


def _text_head(nc, psum, ipool, img, imgT, eye_sb, tpt_sb, textL, b):
    """imgT = img[b].T via PE transpose, then textL rows = tpT.T @ imgT."""
    for tp in range(TIMG // 128):
        imgp = ipool.tile([128, D], f32, name="imgp")
        nc.gpsimd.dma_start(out=imgp, in_=img[b, tp * 128 : (tp + 1) * 128, :])
        pst = psum.tile([128, 512], f32, name="pst", tag="ps")
        for j in range(4):
            nc.tensor.transpose(
                pst[:, j * 128 : (j + 1) * 128],
                imgp[:, j * 128 : (j + 1) * 128],
                eye_sb,
            )
        dst = imgT[:, :, tp * 128 : (tp + 1) * 128]
        srcv = pst.rearrange("p (j c) -> p j c", j=4)
        if tp % 2 == 0:
            nc.vector.tensor_copy(dst, srcv)
        else:
            nc.scalar.copy(dst, srcv)
    r0 = ROW[b]
    for nq in range(4):
        pstx = psum.tile([128, 512], f32, name="pstx", tag="ps")
        for k in range(DT):
            nc.tensor.matmul(
                pstx[:C],
                lhsT=tpt_sb[:, k, :],
                rhs=imgT[:, k, nq * 512 : (nq + 1) * 512],
                start=(k == 0),
                stop=(k == DT - 1),
            )
        nc.scalar.copy(textL[r0 : r0 + C, nq * 512 : (nq + 1) * 512], pstx[:C])


_PROG = None


def _build_program():
    global _PROG
    if _PROG is not None:
        return _PROG
    nc = bacc.Bacc("TRN2", target_bir_lowering=False, debug=False)
    io = {}
    io["feat"] = nc.declare_dram_parameter("feat", [BPC, FD, T], f32r, isOutput=False).ap()
    io["img"] = nc.declare_dram_parameter("img", [BPC, TIMG, D], f32, isOutput=False).ap()
    io["w1t"] = nc.declare_dram_parameter("w1t", [KT, 128, O], f32r, isOutput=False).ap()
    io["w2t"] = nc.declare_dram_parameter("w2t", [MT, 128, O], bf16, isOutput=False).ap()
    io["wct"] = nc.declare_dram_parameter("wct", [DT, 128, C], bf16, isOutput=False).ap()
    io["tpt"] = nc.declare_dram_parameter("tpt", [DT, 128, C], f32r, isOutput=False).ap()
    io["bias_pack"] = nc.declare_dram_parameter("bias_pack", [128, 24], f32, isOutput=False).ap()
    io["bc_pad"] = nc.declare_dram_parameter("bc_pad", [128, 1], f32, isOutput=False).ap()
    io["ind_i"] = nc.declare_dram_parameter("ind_i", [128, MT, GROUPS], f32, isOutput=False).ap()
    io["ind_j"] = nc.declare_dram_parameter("ind_j", [GROUPS, MT, 128], f32, isOutput=False).ap()
    io["eye"] = nc.declare_dram_parameter("eye", [128, 128], f32, isOutput=False).ap()
    io["kpack"] = nc.declare_dram_parameter("kpack", [128, 6], f32, isOutput=False).ap()
    io["scores"] = nc.declare_dram_parameter("scores", [128, 2], f32, isOutput=True)
    with tile.TileContext(nc) as tc:
        _body(tc, io)
    nc.compile()
    _PROG = nc
    return nc


def build_in_maps(input_features, masks, text_proto, img_feats, img_masks,
                  W1, b1, g1, beta1, W2, b2, g2, beta2, Wc, bc):
    """Host-side prep: shard activations per core, pack params (replicated)."""
    asf = lambda a: np.ascontiguousarray(a, dtype=np.float32)
    asb = lambda a: np.ascontiguousarray(a.astype(ml_dtypes.bfloat16))

    w1t = asf(np.asarray(W1, np.float32).T.reshape(KT, 128, O))
    w2t = asb(np.asarray(W2, np.float32).T.reshape(MT, 128, O))
    wct = asb(np.asarray(Wc, np.float32).T.reshape(DT, 128, C))
    tpt = asf(np.asarray(text_proto, np.float32)[0].T.reshape(DT, 128, C))

    bias_pack = np.zeros((128, 24), np.float32)
    for i, v in enumerate([b1, g1, beta1, b2, g2, beta2]):
        bias_pack[:, 4 * i : 4 * i + 4] = np.asarray(v, np.float32).reshape(MT, 128).T
    bc_pad = np.zeros((128, 1), np.float32)
    bc_pad[:C, 0] = np.asarray(bc, np.float32)

    p = np.arange(128)
    ind_i = np.zeros((128, MT, GROUPS), np.float32)
    ind_j = np.zeros((GROUPS, MT, 128), np.float32)
    for m in range(MT):
        ind_i[p, m, m * 8 + p // 16] = 1.0 / GN_N
        ind_j[m * 8 + p // 16, m, p] = 1.0
    eye = np.eye(128, dtype=np.float32)

    text_len = np.asarray(img_masks, np.float32).sum(-1).astype(np.int64)
    cls_len = np.asarray(masks, np.float32).sum((-2, -1)).astype(np.int64)
    k_text = np.maximum(1, text_len // R_ACT)
    k_cls = np.maximum(1, cls_len // R_ACT)

    in_maps = []
    for c in range(NCORES):
        bb = (BPC * c, BPC * c + 1)
        kpack = np.zeros((128, 6), np.float32)
        kpack[:, [0, 3]] = 256.0
        kpack[:, [1, 4]] = 2 * 256.0 - ACT_COLS
        kpack[:, [2, 5]] = 1.0 / 256.0
        for i, b_ in enumerate(bb):
            r = ROW[i]
            kpack[r : r + C, 0] = k_text[b_]
            kpack[r : r + C, 1] = 2.0 * k_text[b_] - ACT_COLS
            kpack[r : r + C, 2] = 1.0 / k_text[b_]
            kpack[r : r + C, 3] = k_cls[b_]
            kpack[r : r + C, 4] = 2.0 * k_cls[b_] - ACT_COLS
            kpack[r : r + C, 5] = 1.0 / k_cls[b_]
        in_maps.append({
            "feat": asf(input_features[bb[0] : bb[1] + 1]),
            "img": asf(img_feats[bb[0] : bb[1] + 1]),
            "w1t": w1t, "w2t": w2t, "wct": wct, "tpt": tpt,
            "bias_pack": bias_pack, "bc_pad": bc_pad,
            "ind_i": ind_i, "ind_j": ind_j, "eye": eye,
            "kpack": kpack,
        })
    return in_maps


def assemble_output(results):
    out = np.zeros((2, B, C), np.float32)
    for c in range(NCORES):
        s = np.asarray(results[c]["scores"]).reshape(128, 2)
        for i in range(BPC):
            r = ROW[i]
            out[0, BPC * c + i] = s[r : r + C, 0]
            out[1, BPC * c + i] = s[r : r + C, 1]
    return out


def _numpy_reference(input_features, masks, text_proto, img_feats, img_masks,
                     W1, b1, g1, beta1, W2, b2, g2, beta2, Wc, bc):
    """Exact numpy fallback, used only if masks are not all-ones."""
    def gn(x, gamma, beta):
        b_, c_, t_ = x.shape
        xr = x.reshape(b_, GROUPS, c_ // GROUPS, t_)
        mu = xr.mean(axis=(2, 3), keepdims=True)
        var = xr.var(axis=(2, 3), keepdims=True)
        xn = ((xr - mu) / np.sqrt(var + EPS)).reshape(b_, c_, t_)
        return xn * gamma[None, :, None] + beta[None, :, None]

    def topk_mean(logits, valid_len):
        vals = -np.sort(-logits, axis=1)
        csum = np.cumsum(vals, axis=1)
        k = np.maximum(1, valid_len // R_ACT).astype(np.int64)
        sel = np.take_along_axis(csum, (k - 1)[:, None, None].repeat(C, 2), axis=1)[:, 0, :]
        return sel / k[:, None]

    x = np.einsum("of,bft->bot", W1, input_features) + b1[None, :, None]
    x = gn(x, g1, beta1) * masks
    x = np.einsum("oc,bct->bot", W2, x) + b2[None, :, None]
    x = gn(x, g2, beta2) * masks
    fe = x.transpose(0, 2, 1)
    cls_logits = 1.0 / (1.0 + np.exp(-(np.einsum("bto,co->btc", fe, Wc) + bc)))
    tp = text_proto[0].T
    text_logits = np.einsum("btd,dc->btc", img_feats, tp)
    text_len = img_masks.sum(-1).astype(np.int64)
    cls_len = masks.sum((-2, -1)).astype(np.int64)
    return np.stack([
        topk_mean(text_logits, text_len),
        topk_mean(cls_logits, cls_len),
    ]).astype(np.float32)


def kernel(**inputs):
    inputs = {k: np.asarray(v) for k, v in inputs.items()}
    masks = inputs["masks"]
    img_masks = inputs["img_masks"]
    if not (np.all(masks == 1.0) and np.all(img_masks == 1.0)):
        # masked GN/logits differ when masks are non-trivial; use exact host path
        return _numpy_reference(**{k: v.astype(np.float32) for k, v in inputs.items()})
    nc = _build_program()
    in_maps = build_in_maps(**inputs)
    res = run_bass_kernel_spmd(nc, in_maps, list(range(NCORES)))
    return assemble_output(res.results)


if __name__ == "__main__":
    import jax
    import reference
    with jax.default_device(jax.devices("cpu")[0]):
        inp = {k: np.asarray(v) for k, v in reference.setup_inputs().items()}
        exp = np.asarray(reference.reference(**inp))
    act = kernel(**inp)
    err = np.abs(act - exp).max() / (np.abs(exp).max() + 1e-12)
    print("max abs err:", np.abs(act - exp).max(), "rel:", err)



# revision 7
# speedup vs baseline: 1.5257x; 1.5257x over previous
"""Trainium2 Bass kernel for nn_ClipForegroundEstimator.

Pipeline (per batch): two (1x1conv -> GroupNorm) blocks over [Fd,T] features,
a sigmoid classifier head, a text-prototype head over img_feats, and a
per-(batch, class) mean of the top-k values along T for both heads.

Sharding: data-parallel over batch. 8 cores x 2 batches each. All params
replicated. Each core returns a [128,2] score tile: col 0 = text head,
col 1 = cls head, with batch b0/b1 at partition offsets 0/32.

Numerics: layer1 and layer2 matmuls run in fp8 e4m3 with DoubleRow perf
mode (2 k-tiles of 128 per instruction, 2x PE throughput). Weights are
host-scaled by SW=64 to stay in e4m3's normal range; layer1 activations are
staged fp8 at 4x their true scale (x1p = 4*x1).  GroupNorm is folded into
the next layer's weights: the fold constants (rs = 1/std, per-group) absorb
all staging scales exactly, so only eps needs compile-time rescaling.
Layer2 output x2p is staged bf16 at true scale; the cls head runs bf16.
Text head: img is DMA-transposed (xbar) straight from DRAM into SBUF bf16
[D, T] layout -- no PE transposes, no PSUM staging.

Top-k mean is computed without sorting: binary-search a per-series threshold
t with count(x > t) == k, then topk_sum = k*t + sum(relu(x - t)); exact for
any t in [x_(k+1), x_(k)], and membership mistakes within the final search
interval contribute only O(interval^2 * local_density) error. The cls search
(the only exposed one) runs all-DVE on bf16 logits, counting a 1024-column
sample; the final relu-sum over the full row corrects the threshold error.

Schedule: the two batches' pipelines are interleaved so the PE never idles
through GroupNorm stat chains -- b1's layer1 superblocks fill b0's GN1/GN2
windows, b0's cls head fills b1's GN windows, and both text heads + the text
top-k search are emitted inside b0's layer1 stream.
"""

import numpy as np
import ml_dtypes

import concourse.bass as bass
import concourse.tile as tile
from concourse import bacc, mybir
from concourse.bass_utils import run_bass_kernel_spmd

f32 = mybir.dt.float32
bf16 = mybir.dt.bfloat16
f8 = mybir.dt.float8e4
AL = mybir.AluOpType
AF = mybir.ActivationFunctionType
AX = mybir.AxisListType
PM = mybir.MatmulPerfMode

# problem shapes (hardcoded per spec)
B, FD, T, O, TIMG, D, C = 16, 2048, 2048, 512, 2048, 512, 20
GROUPS, R_ACT, EPS = 32, 8, 1e-5
NCORES, BPC = 8, 2        # cores, batches per core
GP = FD // 256            # 8 doublerow k-pairs for layer1
GPP = GP // 2             # 4 fetch groups (2 pairs each)
MT = O // 128             # 4 m-tiles of output channels
DT = D // 128             # 4 k-tiles for D contraction
NSB = 4                   # T superblocks of 512
GN_N = (O // GROUPS) * T  # elements per group = 16*2048
S1A = 4.0                 # x1p = S1A * x1_true (fp8 staging scale)
SW = 64.0                 # fp8 weight scale (host)
T_ITERS = 11              # text search iterations (fully hidden)
C_ITERS = 8               # cls search iterations (exposed tail)
C_SAMPLE = 1024           # cls search count sample width

# partition rows of batches inside [128, T] logits tiles
ROW = (0, 32)
DEBUG = False


def _search(nc, sv, junk, junkA, logits, kp_tgt, kv, ki, out_col,
            n_iters, sample, unit_range):
    """Generator: emits the threshold binary search in parts (yield between
    iteration groups so callers can interleave other same-engine work).

    All iteration work is DVE-only: count #(x > mid) over logits[:, :sample]
    (bf16, 4x mode), then a 3-op state update. Final pass: exact-ish
    topk_sum = k*lo + sum(relu(x - lo)) over the full row, split DVE/ACT.
    sv: [128, 8] f32 state tile. junk: [128, T] bf16 scratch (DVE outputs).
    junkA: [128, T//2] bf16 scratch (ACT relu output).
    """
    mid, hw, cnt, t1 = sv[:, 0:1], sv[:, 1:2], sv[:, 2:3], sv[:, 3:4]
    nm, s1, s2, tmp = sv[:, 4:5], sv[:, 5:6], sv[:, 6:7], sv[:, 7:8]
    if unit_range:
        nc.vector.memset(hw, 0.5)
        nc.vector.memset(mid, 0.5)
    else:
        # init range from a 512-col sample: max >= threshold w.p. ~1
        nc.vector.tensor_reduce(s1, logits[:, :512], AX.X, AL.min)
        nc.vector.tensor_reduce(s2, logits[:, :512], AX.X, AL.max)
        nc.vector.tensor_tensor(hw, s2, s1, AL.subtract)
        nc.vector.tensor_scalar(hw, hw, 0.5, None, op0=AL.mult)
        nc.vector.tensor_tensor(mid, s2, hw, AL.subtract)
    done = 0
    while done < n_iters:
        burst = min(3, n_iters - done)
        for _ in range(burst):
            nc.vector.tensor_scalar(
                junk[:, :sample], logits[:, :sample], mid, None,
                op0=AL.is_gt, op1=AL.add, accum_out=cnt,
            )
            nc.vector.tensor_scalar(t1, cnt, kp_tgt, hw, op0=AL.is_ge, op1=AL.mult)
            nc.vector.tensor_scalar(hw, hw, 0.5, None, op0=AL.mult)
            nc.vector.scalar_tensor_tensor(mid, mid, t1, hw, op0=AL.add, op1=AL.subtract)
        done += burst
        yield
    lo = mid
    h = T // 2
    nc.vector.tensor_scalar(nm, lo, -1.0, None, op0=AL.mult)
    nc.vector.tensor_scalar(junk[:, :h], logits[:, :h], lo, None, op0=AL.subtract)
    nc.vector.tensor_scalar(junk[:, :h], junk[:, :h], 0.0, None,
                            op0=AL.max, op1=AL.add, accum_out=s1)
    nc.scalar.activation(junkA, logits[:, h:], AF.Relu, bias=nm, accum_out=s2)
    nc.vector.tensor_tensor(tmp, s1, s2, AL.add)
    nc.vector.scalar_tensor_tensor(tmp, lo, kv, tmp, op0=AL.mult, op1=AL.add)
    nc.vector.tensor_tensor(out_col, tmp, ki, AL.mult)
    yield


def _body(tc, io):
    nc = tc.nc
    feat, img = io["feat"], io["img"]
    w1t, w2t, wct, tpt = io["w1t"], io["w2t"], io["wct"], io["tpt"]
    bias_pack, bc_pad = io["bias_pack"], io["bc_pad"]
    ind_i, ind_j = io["ind_i"], io["ind_j"]
    kpack, scores = io["kpack"], io["scores"]

    import contextlib
    ctx = contextlib.ExitStack()
    with ctx:
        cpool = ctx.enter_context(tc.tile_pool(name="consts", bufs=1))
        fpool = ctx.enter_context(tc.tile_pool(name="fstream", bufs=6))
        xpool = ctx.enter_context(tc.tile_pool(name="acts", bufs=1))
        spool = ctx.enter_context(tc.tile_pool(name="stats", bufs=2))
        scpool = ctx.enter_context(tc.tile_pool(name="scratch", bufs=3))
        bigpool = ctx.enter_context(tc.tile_pool(name="bigs", bufs=1))
        pa = ctx.enter_context(tc.tile_pool(name="pa", bufs=6, space="PSUM"))
        pb = ctx.enter_context(tc.tile_pool(name="pb", bufs=2, space="PSUM"))

        # ---- persistent constants ----
        w1t_sb = cpool.tile([128, GP, 2, O], f8, name="w1t_sb")
        nc.gpsimd.dma_start(out=w1t_sb, in_=w1t)
        w2t_sb = cpool.tile([128, MT, O], bf16, name="w2t_sb")
        nc.gpsimd.dma_start(out=w2t_sb, in_=w2t)
        wct_sb = cpool.tile([128, DT, C], bf16, name="wct_sb")
        nc.gpsimd.dma_start(out=wct_sb, in_=wct)
        tpt_sb = cpool.tile([128, DT, C], bf16, name="tpt_sb")
        nc.gpsimd.dma_start(out=tpt_sb, in_=tpt)
        bp_sb = cpool.tile([128, 32], f32, name="bp_sb")
        nc.gpsimd.dma_start(out=bp_sb, in_=bias_pack)
        bc_sb = cpool.tile([128, 1], f32, name="bc_sb")
        nc.gpsimd.dma_start(out=bc_sb, in_=bc_pad)
        indi_sb = cpool.tile([128, MT, GROUPS], f32, name="indi_sb")
        nc.gpsimd.dma_start(out=indi_sb, in_=ind_i)
        indj_sb = cpool.tile([128, MT, 128], f32, name="indj_sb")
        nc.gpsimd.dma_start(out=indj_sb[:GROUPS], in_=ind_j)
        kp_sb = cpool.tile([128, 6], f32, name="kp_sb")
        nc.gpsimd.dma_start(out=kp_sb, in_=kpack)
        eps_sb = cpool.tile([128, 1], f32, name="eps_sb")
        nc.vector.memset(eps_sb, EPS)

        def bcol(base, m):
            return bp_sb[:, base + m : base + m + 1]

        # ---- big activation / logits tiles ----
        imgT = [bigpool.tile([128, DT, TIMG], bf16, name=f"imgT{b}")
                for b in range(BPC)]
        x1p = [bigpool.tile([128, MT, T], f8, name=f"x1p{b}") for b in range(BPC)]
        x2p = [bigpool.tile([128, MT, T], bf16, name=f"x2p{b}") for b in range(BPC)]
        textL = bigpool.tile([128, T], bf16, name="textL")
        clsL = bigpool.tile([128, T], bf16, name="clsL")
        junk = bigpool.tile([128, T], bf16, name="junk")
        junkA = bigpool.tile([128, T // 2], bf16, name="junkA")
        scout = bigpool.tile([128, 2], f32, name="scout")
        warm = bigpool.tile([1, 1], f32, name="warm")

        w2ts = [bigpool.tile([128, MT, O], f8, name=f"w2ts{b}") for b in range(BPC)]
        wcts = [bigpool.tile([128, DT, C], bf16, name=f"wcts{b}") for b in range(BPC)]
        scq1 = [spool.tile([128, MT, NSB, 2], f32, name=f"scq1b{b}", bufs=1)
                for b in range(BPC)]
        scq2 = [spool.tile([128, MT, NSB, 2], f32, name=f"scq2b{b}", bufs=1)
                for b in range(BPC)]
        bias2 = [spool.tile([128, MT], f32, name=f"bias2b{b}", bufs=1)
                 for b in range(BPC)]
        ngb1 = [spool.tile([128, MT], bf16, name=f"ngb1b{b}", bufs=1)
                for b in range(BPC)]
        ngb2 = [spool.tile([128, MT], bf16, name=f"ngb2b{b}", bufs=1)
                for b in range(BPC)]
        clsb = [spool.tile([128, 1], f32, name=f"clsbb{b}", bufs=1)
                for b in range(BPC)]
        pcbs1 = [spool.tile([128, 2 * MT], f32, name=f"pcbs1b{b}", bufs=1)
                 for b in range(BPC)]
        pcbs2 = [spool.tile([128, 2 * MT], f32, name=f"pcbs2b{b}", bufs=1)
                 for b in range(BPC)]
        sv_t = spool.tile([128, 8], f32, name="sv_t", bufs=1)
        sv_c = spool.tile([128, 8], f32, name="sv_c", bufs=1)

        nc.vector.memset(textL, 0.0)
        nc.vector.memset(clsL, 0.0)
        nc.vector.memset(warm, 0.0)

        # ---- img DMA-transposes: b0 on the scalar queue (idle early),
        # b1 spread over the sync queue between superblocks ----
        for k in range(DT):
            nc.scalar.dma_start_transpose(
                out=imgT[0][:, k, :], in_=img[0, :, k * 128 : (k + 1) * 128]
            )

        # ------------------------------------------------------------------
        # emission helpers
        # ------------------------------------------------------------------
        def l1_superblock(b, nsb, gp_range):
            """Layer1 MMs for one superblock's fetch-group range. The m2/m3
            accumulators defer their gp0 contribution to the end so their
            PSUM slots (reused from the previous superblock) have time to
            drain before their first write."""
            ns0 = nsb * 512
            for gp in gp_range:
                ftp = fpool.tile([128, 2, 2, 512], f8, name="ftp", tag="ftp")
                nc.sync.dma_start(
                    out=ftp, in_=feat[b, :, 2 * gp : 2 * gp + 2, :, ns0 : ns0 + 512]
                )
                for q in range(2):
                    g = 2 * gp + q
                    for m in range(MT):
                        if gp == 0 and m >= 2:
                            continue  # deferred below
                        nc.tensor.matmul(
                            _l1ps(b, nsb)[m],
                            lhsT=w1t_sb[:, g, :, m * 128 : (m + 1) * 128],
                            rhs=ftp[:, q, :, :],
                            start=(g == 0) if m < 2 else (g == 2),
                            stop=(g == GP - 1) if m < 2 else False,
                            perf_mode=PM.DoubleRow,
                        )
                if gp == 0:
                    _l1_gp0_tiles[(b, nsb)] = ftp

        _l1_psum = {}
        _l1_gp0_tiles = {}

        def _l1ps(b, nsb):
            key = (b, nsb)
            if key not in _l1_psum:
                _l1_psum[key] = [
                    pa.tile([128, 512], f32, name=f"pa{m}", tag="pa") for m in range(MT)
                ]
            return _l1_psum[key]

        def l1_finish(b, nsb):
            """Deferred gp0 MMs for m2/m3 (their stop), then drains."""
            ns0 = nsb * 512
            ftp = _l1_gp0_tiles.pop((b, nsb))
            ps = _l1_psum.pop((b, nsb))
            for q in range(2):
                for m in (2, 3):
                    nc.tensor.matmul(
                        ps[m],
                        lhsT=w1t_sb[:, q, :, m * 128 : (m + 1) * 128],
                        rhs=ftp[:, q, :, :],
                        start=False,
                        stop=(q == 1),
                        perf_mode=PM.DoubleRow,
                    )
            for m in range(MT):
                xs = x1p[b][:, m, ns0 : ns0 + 512]
                # x1p = ps/16 + 4*b1  (= 4 * x1_true); S1 accumulated on ACT
                nc.scalar.activation(
                    xs, ps[m], AF.Identity, bias=bcol(0, m), scale=S1A / SW,
                    accum_out=scq1[b][:, m, nsb, 0:1],
                )
                sqs = scpool.tile([128, 512], bf16, name="sqs", tag="sqs")
                nc.vector.scalar_tensor_tensor(
                    sqs, xs, 1.0, xs, op0=AL.bypass, op1=AL.mult,
                    accum_out=scq1[b][:, m, nsb, 1:2],
                )

        def text_mms(b):
            r0 = ROW[b]
            for nq in range(4):
                pstx = pb.tile([128, 512], f32, name="pstx", tag="pb")
                for k in range(DT):
                    nc.tensor.matmul(
                        pstx[:C],
                        lhsT=tpt_sb[:, k, :],
                        rhs=imgT[b][:, k, nq * 512 : (nq + 1) * 512],
                        start=(k == 0),
                        stop=(k == DT - 1),
                    )
                nc.vector.tensor_copy(textL[r0 : r0 + C, nq * 512 : (nq + 1) * 512],
                                      pstx[:C])

        def gn_stats(b, scq, pcbs, var_scale, lname):
            """GN group stats: psg MM (PE) -> grp chain (DVE/ACT) -> pcb MM
            (PE) -> pcbs copy (DVE). var_scale: staging scale s with
            staged stats (S1 = s*sum x, S2 = s^2*sum x^2)."""
            psg = pb.tile([128, 512], f32, name=f"psg_{lname}", tag="pb")
            for m in range(MT):
                nc.tensor.matmul(
                    psg[:GROUPS, 0 : 2 * NSB],
                    lhsT=indi_sb[:, m, :],
                    rhs=scq[:, m].rearrange("p a b -> p (a b)"),
                    start=(m == 0),
                    stop=(m == MT - 1),
                )
            grp = spool.tile([128, 4], f32, name=f"grp_{lname}", bufs=1)
            nc.vector.tensor_reduce(
                grp[:GROUPS, 0:2],
                psg[:GROUPS, 0 : 2 * NSB].rearrange("p (j s) -> p s j", j=NSB),
                AX.X, AL.add,
            )
            # grp cols: 0 = mu' (= s*mu), 1 = msq' (= s^2*msq)
            # mu'^2 - msq' = -s^2 * var ;  std = sqrt(-(x)/s^2 + eps)
            nc.vector.scalar_tensor_tensor(
                grp[:GROUPS, 2:3], grp[:GROUPS, 0:1], grp[:GROUPS, 0:1],
                grp[:GROUPS, 1:2], op0=AL.mult, op1=AL.subtract,
            )
            nc.scalar.activation(
                grp[:GROUPS, 2:3], grp[:GROUPS, 2:3], AF.Sqrt,
                bias=eps_sb[:GROUPS], scale=-1.0 / (var_scale * var_scale),
            )
            nc.vector.reciprocal(grp[:GROUPS, 2:3], grp[:GROUPS, 2:3])  # true rs
            nc.vector.tensor_tensor(
                grp[:GROUPS, 3:4], grp[:GROUPS, 2:3], grp[:GROUPS, 0:1], AL.mult
            )  # rs * mu'
            pcb = pb.tile([128, 512], f32, name=f"pcb_{lname}", tag="pb")
            for m in range(MT):
                nc.tensor.matmul(
                    pcb[:, 2 * m : 2 * m + 2],
                    lhsT=indj_sb[:GROUPS, m, :],
                    rhs=grp[:GROUPS, 2:4],
                    start=True,
                    stop=True,
                )
            nc.vector.tensor_copy(pcbs, pcb[:, 0 : 2 * MT])

        def fold1(b):
            # w2ts = W2^T * rs * (16*gamma1): with x1p = 4*x1 and SW=64 this
            # makes ps2 = 64 * W2 @ (gamma1*rs*x1)
            for k in range(MT):
                nc.vector.tensor_scalar(
                    w2ts[b][:, k, :], w2t_sb[:, k, :],
                    pcbs1[b][:, 2 * k : 2 * k + 1], bcol(4, k),
                    op0=AL.mult, op1=AL.mult,
                )
                # ngb1 = beta1 - gamma1*rs*mu   (gamma1n = -gamma1/4 hosted)
                nc.vector.tensor_scalar(
                    ngb1[b][:, k : k + 1], pcbs1[b][:, 2 * k + 1 : 2 * k + 2],
                    bcol(8, k), bcol(12, k), op0=AL.mult, op1=AL.add,
                )

        def psb_mm(b):
            psb = pb.tile([128, 512], f32, name=f"psb{b}", tag="pb")
            for m in range(MT):
                for k in range(MT):
                    nc.tensor.matmul(
                        psb[:, m : m + 1],
                        lhsT=w2t_sb[:, k, m * 128 : (m + 1) * 128],
                        rhs=ngb1[b][:, k : k + 1],
                        start=(k == 0),
                        stop=(k == MT - 1),
                    )
            # bias2 = b2 + W2 @ (beta1 - gamma1*rs*mu)
            nc.vector.tensor_tensor(bias2[b], bp_sb[:, 16:20], psb[:, 0:MT], AL.add)

        def l2_chunk(b, m, nsb):
            ns0 = nsb * 512
            ps2 = pb.tile([128, 512], f32, name="ps2", tag="pb")
            for q in range(2):
                nc.tensor.matmul(
                    ps2,
                    lhsT=w2ts[b][:, 2 * q : 2 * q + 2, m * 128 : (m + 1) * 128],
                    rhs=x1p[b][:, 2 * q : 2 * q + 2, ns0 : ns0 + 512],
                    start=(q == 0),
                    stop=(q == 1),
                    perf_mode=PM.DoubleRow,
                )
            xs = x2p[b][:, m, ns0 : ns0 + 512]
            nc.scalar.activation(
                xs, ps2, AF.Identity, bias=bias2[b][:, m : m + 1], scale=1.0 / SW,
                accum_out=scq2[b][:, m, nsb, 0:1],
            )
            sqs = scpool.tile([128, 512], bf16, name="sqs", tag="sqs")
            nc.vector.scalar_tensor_tensor(
                sqs, xs, 1.0, xs, op0=AL.bypass, op1=AL.mult,
                accum_out=scq2[b][:, m, nsb, 1:2],
            )

        def fold2(b):
            for k in range(DT):
                nc.vector.tensor_scalar(
                    wcts[b][:, k, :], wct_sb[:, k, :],
                    pcbs2[b][:, 2 * k : 2 * k + 1], bcol(20, k),
                    op0=AL.mult, op1=AL.mult,
                )
                nc.vector.tensor_scalar(
                    ngb2[b][:, k : k + 1], pcbs2[b][:, 2 * k + 1 : 2 * k + 2],
                    bcol(24, k), bcol(28, k), op0=AL.mult, op1=AL.add,
                )

        def pscb_mm(b):
            pscb = pb.tile([128, 512], f32, name=f"pscb{b}", tag="pb")
            for k in range(DT):
                nc.tensor.matmul(
                    pscb[:C, 0:1],
                    lhsT=wct_sb[:, k, :],
                    rhs=ngb2[b][:, k : k + 1],
                    start=(k == 0),
                    stop=(k == DT - 1),
                )
            nc.vector.tensor_tensor(clsb[b][:C], bc_sb[:C], pscb[:C, 0:1], AL.add)

        def cls_mms(b, nqs):
            r0 = ROW[b]
            for nq in nqs:
                psc = pb.tile([128, 512], f32, name="psc", tag="pb")
                for k in range(MT):
                    nc.tensor.matmul(
                        psc[:C],
                        lhsT=wcts[b][:, k, :],
                        rhs=x2p[b][:, k, nq * 512 : (nq + 1) * 512],
                        start=(k == 0),
                        stop=(k == MT - 1),
                    )
                nc.scalar.activation(
                    clsL[r0 : r0 + C, nq * 512 : (nq + 1) * 512],
                    psc[:C], AF.Sigmoid, bias=clsb[b][:C],
                )

        def b1_transpose(k):
            nc.sync.dma_start_transpose(
                out=imgT[1][:, k, :], in_=img[1, :, k * 128 : (k + 1) * 128]
            )

        # ------------------------------------------------------------------
        # schedule
        # ------------------------------------------------------------------
        # L1(b0) with text heads woven in
        l1_superblock(0, 0, range(GPP)); l1_finish(0, 0)
        b1_transpose(0)
        l1_superblock(0, 1, range(GPP)); l1_finish(0, 1)
        b1_transpose(1)
        l1_superblock(0, 2, range(GPP)); l1_finish(0, 2)
        text_mms(0)
        b1_transpose(2)
        l1_superblock(0, 3, range(GPP)); l1_finish(0, 3)
        b1_transpose(3)
        text_mms(1)
        tsearch = _search(nc, sv_t, junk, junkA, textL,
                          kp_sb[:, 0:1], kp_sb[:, 1:2], kp_sb[:, 2:3],
                          scout[:, 0:1], n_iters=T_ITERS, sample=T,
                          unit_range=False)
        next(tsearch)  # init + iters 0-2

        # L1(b1) sb0 covers GN1(b0)
        l1_superblock(1, 0, range(GPP)); l1_finish(1, 0)
        gn_stats(0, scq1[0], pcbs1[0], S1A, "gn1b0")
        next(tsearch)  # iters 3-5

        l1_superblock(1, 1, range(2))
        fold1(0)
        l1_superblock(1, 1, range(2, GPP)); l1_finish(1, 1)
        psb_mm(0)
        next(tsearch)  # iters 6-8

        l1_superblock(1, 2, range(2))
        l2_chunk(0, 0, 0); l2_chunk(0, 1, 0)
        l1_superblock(1, 2, range(2, GPP)); l1_finish(1, 2)
        l2_chunk(0, 2, 0); l2_chunk(0, 3, 0)
        next(tsearch)  # iters 9-10
        l1_superblock(1, 3, range(2))
        l2_chunk(0, 0, 1); l2_chunk(0, 1, 1)
        l1_superblock(1, 3, range(2, GPP)); l1_finish(1, 3)
        next(tsearch)  # final relu-sum + combine
        l2_chunk(0, 2, 1); l2_chunk(0, 3, 1)
        l2_chunk(0, 0, 2); l2_chunk(0, 1, 2)
        gn_stats(1, scq1[1], pcbs1[1], S1A, "gn1b1")
        l2_chunk(0, 2, 2); l2_chunk(0, 3, 2)
        l2_chunk(0, 0, 3); l2_chunk(0, 1, 3)
        fold1(1)
        l2_chunk(0, 2, 3); l2_chunk(0, 3, 3)
        psb_mm(1)
        gn_stats(0, scq2[0], pcbs2[0], 1.0, "gn2b0")
        fold2(0)
        pscb_mm(0)
        # CLS(b0) split across b1's GN windows / L2(b1)
        cls_mms(0, [0, 1])
        for nsb in range(2):
            for m in range(MT):
                l2_chunk(1, m, nsb)
        cls_mms(0, [2])
        for nsb in range(2, NSB):
            for m in range(MT):
                l2_chunk(1, m, nsb)
        cls_mms(0, [3])
        gn_stats(1, scq2[1], pcbs2[1], 1.0, "gn2b1")
        fold2(1)
        # pre-warm the sigmoid activation table while CLS(b1) MMs run
        nc.scalar.activation(warm, warm, AF.Sigmoid)
        pscb_mm(1)
        cls_mms(1, [0, 1, 2, 3])
        csearch = _search(nc, sv_c, junk, junkA, clsL,
                          kp_sb[:, 3:4], kp_sb[:, 4:5], kp_sb[:, 5:6],
                          scout[:, 1:2], n_iters=C_ITERS, sample=C_SAMPLE,
                          unit_range=True)
        for _ in csearch:
            pass
        nc.sync.dma_start(out=scores.ap(), in_=scout)
        if DEBUG:
            nc.sync.dma_start(out=io["dbg_text"].ap(), in_=textL)
            nc.sync.dma_start(out=io["dbg_cls"].ap(), in_=clsL)
            nc.sync.dma_start(out=io["dbg_sv"].ap(), in_=sv_c)


_PROG = None


def _build_program():
    global _PROG
    if _PROG is not None:
        return _PROG
    nc = bacc.Bacc("TRN2", target_bir_lowering=False, debug=False)
    io = {}
    io["feat"] = nc.declare_dram_parameter("feat", [BPC, 128, GP, 2, T], f8, isOutput=False).ap()
    io["img"] = nc.declare_dram_parameter("img", [BPC, TIMG, D], bf16, isOutput=False).ap()
    io["w1t"] = nc.declare_dram_parameter("w1t", [128, GP, 2, O], f8, isOutput=False).ap()
    io["w2t"] = nc.declare_dram_parameter("w2t", [128, MT, O], bf16, isOutput=False).ap()
    io["wct"] = nc.declare_dram_parameter("wct", [128, DT, C], bf16, isOutput=False).ap()
    io["tpt"] = nc.declare_dram_parameter("tpt", [128, DT, C], bf16, isOutput=False).ap()
    io["bias_pack"] = nc.declare_dram_parameter("bias_pack", [128, 32], f32, isOutput=False).ap()
    io["bc_pad"] = nc.declare_dram_parameter("bc_pad", [128, 1], f32, isOutput=False).ap()
    io["ind_i"] = nc.declare_dram_parameter("ind_i", [128, MT, GROUPS], f32, isOutput=False).ap()
    io["ind_j"] = nc.declare_dram_parameter("ind_j", [GROUPS, MT, 128], f32, isOutput=False).ap()
    io["kpack"] = nc.declare_dram_parameter("kpack", [128, 6], f32, isOutput=False).ap()
    io["scores"] = nc.declare_dram_parameter("scores", [128, 2], f32, isOutput=True)
    if DEBUG:
        io["dbg_text"] = nc.declare_dram_parameter("dbg_text", [128, T], bf16, isOutput=True)
        io["dbg_cls"] = nc.declare_dram_parameter("dbg_cls", [128, T], bf16, isOutput=True)
        io["dbg_sv"] = nc.declare_dram_parameter("dbg_sv", [128, 8], f32, isOutput=True)
    with tile.TileContext(nc) as tc:
        _body(tc, io)
    nc.compile()
    _PROG = nc
    return nc


def build_in_maps(input_features, masks, text_proto, img_feats, img_masks,
                  W1, b1, g1, beta1, W2, b2, g2, beta2, Wc, bc):
    """Host-side prep: shard activations per core, pack params (replicated)."""
    asf = lambda a: np.ascontiguousarray(a, dtype=np.float32)
    asb = lambda a: np.ascontiguousarray(np.asarray(a, np.float32).astype(ml_dtypes.bfloat16))
    as8 = lambda a: np.ascontiguousarray(np.asarray(a, np.float32).astype(ml_dtypes.float8_e4m3))

    # feat: [B, FD, T] -> [B, 128(p), GP(g), 2(j), T] with FD = 256g+128j+p
    featdr = as8(np.asarray(input_features, np.float32)
                 .reshape(B, GP, 2, 128, T).transpose(0, 3, 1, 2, 4))
    imgb = asb(img_feats)

    # w1t: [128(p), GP, 2, O] with w1t[p,g,j,o] = 64*W1[o, 256g+128j+p]
    w1s = np.asarray(W1, np.float32).T * SW          # [FD, O]
    w1dr = as8(w1s.reshape(GP, 2, 128, O).transpose(2, 0, 1, 3))
    w2tb = asb(np.asarray(W2, np.float32).T.reshape(MT, 128, O).transpose(1, 0, 2))
    wctb = asb(np.asarray(Wc, np.float32).T.reshape(DT, 128, C).transpose(1, 0, 2))
    tptb = asb(np.asarray(text_proto, np.float32)[0].T.reshape(DT, 128, C).transpose(1, 0, 2))

    bias_pack = np.zeros((128, 32), np.float32)
    cols = [
        np.asarray(b1, np.float32) * S1A,       # 0-3: 4*b1 (drain bias)
        np.asarray(g1, np.float32) * 16.0,      # 4-7: 16*gamma1 (w2ts fold)
        np.asarray(g1, np.float32) * (-0.25),   # 8-11: -gamma1/4 (ngb1)
        np.asarray(beta1, np.float32),          # 12-15
        np.asarray(b2, np.float32),             # 16-19
        np.asarray(g2, np.float32),             # 20-23
        np.asarray(g2, np.float32) * (-1.0),    # 24-27
        np.asarray(beta2, np.float32),          # 28-31
    ]
    for i, v in enumerate(cols):
        bias_pack[:, 4 * i : 4 * i + 4] = v.reshape(MT, 128).T
    bc_pad = np.zeros((128, 1), np.float32)
    bc_pad[:C, 0] = np.asarray(bc, np.float32)

    p = np.arange(128)
    ind_i = np.zeros((128, MT, GROUPS), np.float32)
    ind_j = np.zeros((GROUPS, MT, 128), np.float32)
    for m in range(MT):
        ind_i[p, m, m * 8 + p // 16] = 1.0 / GN_N
        ind_j[m * 8 + p // 16, m, p] = 1.0

    text_len = np.asarray(img_masks, np.float32).sum(-1).astype(np.int64)
    cls_len = np.asarray(masks, np.float32).sum((-2, -1)).astype(np.int64)
    k_text = np.maximum(1, text_len // R_ACT)
    k_cls = np.maximum(1, cls_len // R_ACT)

    in_maps = []
    for c in range(NCORES):
        bb = (BPC * c, BPC * c + 1)
        kpack = np.zeros((128, 6), np.float32)
        kpack[:, 0] = 256.0
        kpack[:, 1] = 256.0
        kpack[:, 2] = 1.0 / 256.0
        kpack[:, 3] = 256.0 * C_SAMPLE / T
        kpack[:, 4] = 256.0
        kpack[:, 5] = 1.0 / 256.0
        for i, b_ in enumerate(bb):
            r = ROW[i]
            kpack[r : r + C, 0] = k_text[b_]
            kpack[r : r + C, 1] = k_text[b_]
            kpack[r : r + C, 2] = 1.0 / k_text[b_]
            kpack[r : r + C, 3] = np.round(k_cls[b_] * C_SAMPLE / T)
            kpack[r : r + C, 4] = k_cls[b_]
            kpack[r : r + C, 5] = 1.0 / k_cls[b_]
        in_maps.append({
            "feat": featdr[bb[0] : bb[1] + 1],
            "img": imgb[bb[0] : bb[1] + 1],
            "w1t": w1dr, "w2t": w2tb, "wct": wctb, "tpt": tptb,
            "bias_pack": bias_pack, "bc_pad": bc_pad,
            "ind_i": ind_i, "ind_j": ind_j,
            "kpack": kpack,
        })
    return in_maps


def assemble_output(results):
    out = np.zeros((2, B, C), np.float32)
    for c in range(NCORES):
        s = np.asarray(results[c]["scores"]).reshape(128, 2)
        for i in range(BPC):
            r = ROW[i]
            out[0, BPC * c + i] = s[r : r + C, 0]
            out[1, BPC * c + i] = s[r : r + C, 1]
    return out


def _numpy_reference(input_features, masks, text_proto, img_feats, img_masks,
                     W1, b1, g1, beta1, W2, b2, g2, beta2, Wc, bc):
    """Exact numpy fallback, used only if masks are not all-ones."""
    def gn(x, gamma, beta):
        b_, c_, t_ = x.shape
        xr = x.reshape(b_, GROUPS, c_ // GROUPS, t_)
        mu = xr.mean(axis=(2, 3), keepdims=True)
        var = xr.var(axis=(2, 3), keepdims=True)
        xn = ((xr - mu) / np.sqrt(var + EPS)).reshape(b_, c_, t_)
        return xn * gamma[None, :, None] + beta[None, :, None]

    def topk_mean(logits, valid_len):
        vals = -np.sort(-logits, axis=1)
        csum = np.cumsum(vals, axis=1)
        k = np.maximum(1, valid_len // R_ACT).astype(np.int64)
        sel = np.take_along_axis(csum, (k - 1)[:, None, None].repeat(C, 2), axis=1)[:, 0, :]
        return sel / k[:, None]

    x = np.einsum("of,bft->bot", W1, input_features) + b1[None, :, None]
    x = gn(x, g1, beta1) * masks
    x = np.einsum("oc,bct->bot", W2, x) + b2[None, :, None]
    x = gn(x, g2, beta2) * masks
    fe = x.transpose(0, 2, 1)
    cls_logits = 1.0 / (1.0 + np.exp(-(np.einsum("bto,co->btc", fe, Wc) + bc)))
    tp = text_proto[0].T
    text_logits = np.einsum("btd,dc->btc", img_feats, tp)
    text_len = img_masks.sum(-1).astype(np.int64)
    cls_len = masks.sum((-2, -1)).astype(np.int64)
    return np.stack([
        topk_mean(text_logits, text_len),
        topk_mean(cls_logits, cls_len),
    ]).astype(np.float32)


def kernel(**inputs):
    inputs = {k: np.asarray(v) for k, v in inputs.items()}
    masks = inputs["masks"]
    img_masks = inputs["img_masks"]
    if not (np.all(masks == 1.0) and np.all(img_masks == 1.0)):
        # masked GN/logits differ when masks are non-trivial; use exact host path
        return _numpy_reference(**{k: v.astype(np.float32) for k, v in inputs.items()})
    nc = _build_program()
    in_maps = build_in_maps(**inputs)
    res = run_bass_kernel_spmd(nc, in_maps, list(range(NCORES)))
    return assemble_output(res.results)


if __name__ == "__main__":
    import jax
    import reference
    with jax.default_device(jax.devices("cpu")[0]):
        inp = {k: np.asarray(v) for k, v in reference.setup_inputs().items()}
        exp = np.asarray(reference.reference(**inp))
    act = kernel(**inp)
    err = np.abs(act - exp).max() / (np.abs(exp).max() + 1e-12)
    print("max abs err:", np.abs(act - exp).max(), "rel:", err)


# revision 9
# speedup vs baseline: 1.6955x; 1.1113x over previous
"""Trainium2 Bass kernel for nn_ClipForegroundEstimator.

Pipeline (per batch): two (1x1conv -> GroupNorm) blocks over [Fd,T] features,
a sigmoid classifier head, a text-prototype head over img_feats, and a
per-(batch, class) mean of the top-k values along T for both heads.

Sharding: data-parallel over batch. 8 cores x 2 batches each. All params
replicated. Each core returns a [128,2] score tile: col 0 = text head,
col 1 = cls head, with batch b0/b1 at partition offsets 0/32.

Numerics: layer1 and layer2 matmuls run in fp8 e4m3 with DoubleRow perf
mode (2 k-tiles of 128 per instruction, 2x PE throughput). Weights are
host-scaled by SW=64 to stay in e4m3's normal range; layer1 activations are
staged fp8 at 4x their true scale (x1p = 4*x1). GroupNorm is folded into
the next layer's weights; the fold constants (rs = 1/std per group) absorb
all staging scales exactly, so only eps needs compile-time rescaling.
Layer2 output x2p is staged bf16 at true scale; its GN stats come from DVE
bn_stats/bn_aggr (one pass, no ACT accumulator read). The cls head runs
bf16. Text head: img is DMA-transposed (xbar) straight from DRAM into SBUF
bf16 [D, T] layout -- no PE transposes, no PSUM staging. Each transpose
blocks the issuing hwdge queue ~2.4us, so they are spread one-per-
superblock through the sync queue's ft stream.

Top-k mean is computed without sorting: binary-search a per-series threshold
t with count(x > t) == k on a column SAMPLE, then
topk_sum = k*t + sum(relu(x - t)) over the full row; the relu sum makes the
result exact up to O(density * delta^2 / k) for threshold offset delta.
Counts run at DVE 1x (~1.1ns/col; accum_out disables the 4x mode), so
sample width is the main cost knob. The cls searches run per batch with
partition-sliced APs on per-batch logit tiles: batch b1's count sample
(cols 0:512) only needs its first sigmoid chunk, so the search overlaps
the remaining cls matmuls and only the final relu-sum is exposed.

Schedule: the two batches' pipelines are interleaved so the PE never idles
through GroupNorm stat chains -- b1's layer1 superblocks fill b0's GN1/GN2
windows, b0's cls head fills b1's GN windows. ACT Sqrt/Sigmoid table loads
(1.28us each) are pre-warmed by dummy activations carrying real data deps
so the list scheduler cannot hoist them ahead of the eviction point.
"""

import numpy as np
import ml_dtypes

import concourse.bass as bass
import concourse.tile as tile
from concourse import bacc, mybir
from concourse.bass_utils import run_bass_kernel_spmd

f32 = mybir.dt.float32
bf16 = mybir.dt.bfloat16
f8 = mybir.dt.float8e4
AL = mybir.AluOpType
AF = mybir.ActivationFunctionType
AX = mybir.AxisListType
PM = mybir.MatmulPerfMode

# problem shapes (hardcoded per spec)
B, FD, T, O, TIMG, D, C = 16, 2048, 2048, 512, 2048, 512, 20
GROUPS, R_ACT, EPS = 32, 8, 1e-5
NCORES, BPC = 8, 2        # cores, batches per core
GP = FD // 256            # 8 doublerow k-pairs for layer1
GPP = GP // 2             # 4 fetch groups (2 pairs each)
MT = O // 128             # 4 m-tiles of output channels
DT = D // 128             # 4 k-tiles for D contraction
NSB = 4                   # T superblocks of 512
GN_N = (O // GROUPS) * T  # elements per group = 16*2048
S1A = 4.0                 # x1p = S1A * x1_true (fp8 staging scale)
SW = 64.0                 # fp8 weight scale (host)
T_ITERS = 8               # text search iterations (hidden)
T_SAMPLE = 1024           # text search count sample width
C_ITERS = 7               # cls search iterations
C_SAMPLE = 512            # cls search count sample width

# partition rows of batches inside [128, T] logits tiles
ROW = (0, 32)
DEBUG = False


def _search(nc, sv, junk, junkA, logits, kp_tgt, kv, ki, out_col,
            n_iters, sample, unit_range, rows=slice(0, 128)):
    """Generator: emits the threshold binary search in parts (yield between
    iteration groups so callers can interleave other same-engine work).

    All iteration work is DVE-only: count #(x > mid) over
    logits[rows, :sample] (1x, ~1.1ns/col), then a 3-op state update.
    Final pass: topk_sum = k*lo + sum(relu(x - lo)) over the full row,
    split DVE/ACT. All APs are partition-sliced by `rows` so dependencies
    attach only to that batch's logit writes.
    """
    r = rows
    kp_tgt, kv, ki, out_col = kp_tgt[r, :], kv[r, :], ki[r, :], out_col[r, :]
    mid, hw, cnt, t1 = sv[r, 0:1], sv[r, 1:2], sv[r, 2:3], sv[r, 3:4]
    nm, s1, s2, tmp = sv[r, 4:5], sv[r, 5:6], sv[r, 6:7], sv[r, 7:8]
    if unit_range:
        nc.vector.memset(hw, 0.5)
        nc.vector.memset(mid, 0.5)
    else:
        # init range from a 512-col sample: max >= threshold w.p. ~1
        nc.vector.tensor_reduce(s1, logits[r, :512], AX.X, AL.min)
        nc.vector.tensor_reduce(s2, logits[r, :512], AX.X, AL.max)
        nc.vector.tensor_tensor(hw, s2, s1, AL.subtract)
        nc.vector.tensor_scalar(hw, hw, 0.5, None, op0=AL.mult)
        nc.vector.tensor_tensor(mid, s2, hw, AL.subtract)
    done = 0
    while done < n_iters:
        burst = min(3, n_iters - done)
        for _ in range(burst):
            nc.vector.tensor_scalar(
                junk[r, :sample], logits[r, :sample], mid, None,
                op0=AL.is_gt, op1=AL.add, accum_out=cnt,
            )
            nc.vector.tensor_scalar(t1, cnt, kp_tgt, hw, op0=AL.is_ge, op1=AL.mult)
            nc.vector.tensor_scalar(hw, hw, 0.5, None, op0=AL.mult)
            nc.vector.scalar_tensor_tensor(mid, mid, t1, hw, op0=AL.add, op1=AL.subtract)
        done += burst
        yield
    lo = mid
    h = T // 2
    nc.vector.tensor_scalar(nm, lo, -1.0, None, op0=AL.mult)
    nc.vector.tensor_scalar(junk[r, :h], logits[r, :h], lo, None, op0=AL.subtract)
    nc.vector.tensor_scalar(junk[r, :h], junk[r, :h], 0.0, None,
                            op0=AL.max, op1=AL.add, accum_out=s1)
    nc.scalar.activation(junkA[r, :], logits[r, h:], AF.Relu, bias=nm, accum_out=s2)
    nc.vector.tensor_tensor(tmp, s1, s2, AL.add)
    nc.vector.scalar_tensor_tensor(tmp, lo, kv, tmp, op0=AL.mult, op1=AL.add)
    nc.vector.tensor_tensor(out_col, tmp, ki, AL.mult)
    yield


def _body(tc, io):
    nc = tc.nc
    feat, img = io["feat"], io["img"]
    w1t, w2t, wct, tpt = io["w1t"], io["w2t"], io["wct"], io["tpt"]
    bias_pack, bc_pad = io["bias_pack"], io["bc_pad"]
    ind_i, ind_i2, ind_j = io["ind_i"], io["ind_i2"], io["ind_j"]
    kpack, scores = io["kpack"], io["scores"]

    import contextlib
    ctx = contextlib.ExitStack()
    with ctx:
        cpool = ctx.enter_context(tc.tile_pool(name="consts", bufs=1))
        fpool = ctx.enter_context(tc.tile_pool(name="fstream", bufs=8))
        spool = ctx.enter_context(tc.tile_pool(name="stats", bufs=2))
        scpool = ctx.enter_context(tc.tile_pool(name="scratch", bufs=3))
        bigpool = ctx.enter_context(tc.tile_pool(name="bigs", bufs=1))
        pa = ctx.enter_context(tc.tile_pool(name="pa", bufs=6, space="PSUM"))
        pb = ctx.enter_context(tc.tile_pool(name="pb", bufs=2, space="PSUM"))

        # ---- persistent constants ----
        w1t_sb = cpool.tile([128, GP, 2, O], f8, name="w1t_sb")
        nc.gpsimd.dma_start(out=w1t_sb, in_=w1t)
        w2t_sb = cpool.tile([128, MT, O], bf16, name="w2t_sb")
        nc.gpsimd.dma_start(out=w2t_sb, in_=w2t)
        wct_sb = cpool.tile([128, DT, C], bf16, name="wct_sb")
        nc.gpsimd.dma_start(out=wct_sb, in_=wct)
        tpt_sb = cpool.tile([128, DT, C], bf16, name="tpt_sb")
        nc.gpsimd.dma_start(out=tpt_sb, in_=tpt)
        bp_sb = cpool.tile([128, 32], f32, name="bp_sb")
        nc.gpsimd.dma_start(out=bp_sb, in_=bias_pack)
        bc_sb = cpool.tile([128, 1], f32, name="bc_sb")
        nc.gpsimd.dma_start(out=bc_sb, in_=bc_pad)
        indi_sb = cpool.tile([128, MT, GROUPS], f32, name="indi_sb")
        nc.gpsimd.dma_start(out=indi_sb, in_=ind_i)
        indi2_sb = cpool.tile([128, MT, GROUPS], f32, name="indi2_sb")
        nc.gpsimd.dma_start(out=indi2_sb, in_=ind_i2)
        indj_sb = cpool.tile([128, MT, 128], f32, name="indj_sb")
        nc.gpsimd.dma_start(out=indj_sb[:GROUPS], in_=ind_j)
        kp_sb = cpool.tile([128, 6], f32, name="kp_sb")
        nc.gpsimd.dma_start(out=kp_sb, in_=kpack)
        eps_sb = cpool.tile([128, 1], f32, name="eps_sb")
        nc.vector.memset(eps_sb, EPS)

        def bcol(base, m):
            return bp_sb[:, base + m : base + m + 1]

        # ---- big activation / logits tiles ----
        imgT = [bigpool.tile([128, DT, TIMG], bf16, name=f"imgT{b}")
                for b in range(BPC)]
        x1p = [bigpool.tile([128, MT, T], f8, name=f"x1p{b}") for b in range(BPC)]
        x2p = [bigpool.tile([128, MT, T], bf16, name=f"x2p{b}") for b in range(BPC)]
        textL = bigpool.tile([128, T], bf16, name="textL")
        clsL = [bigpool.tile([128, T], bf16, name=f"clsL{b}") for b in range(BPC)]
        junkT = bigpool.tile([128, T], bf16, name="junkT")
        junkAT = bigpool.tile([128, T // 2], bf16, name="junkAT")
        junkC = [bigpool.tile([128, T], bf16, name=f"junkC{b}") for b in range(BPC)]
        junkAC = [bigpool.tile([128, T // 2], bf16, name=f"junkAC{b}")
                  for b in range(BPC)]
        scout = bigpool.tile([128, 2], f32, name="scout")
        warm = bigpool.tile([1, 1], f32, name="warm")

        w2ts = [bigpool.tile([128, MT, O], f8, name=f"w2ts{b}") for b in range(BPC)]
        wcts = [bigpool.tile([128, DT, C], bf16, name=f"wcts{b}") for b in range(BPC)]
        scq1 = [spool.tile([128, MT, NSB, 2], f32, name=f"scq1b{b}", bufs=1)
                for b in range(BPC)]
        bnq2 = [spool.tile([128, MT, NSB, 6], f32, name=f"bnq2b{b}", bufs=1)
                for b in range(BPC)]
        bias2 = [spool.tile([128, MT], f32, name=f"bias2b{b}", bufs=1)
                 for b in range(BPC)]
        ngb1 = [spool.tile([128, MT], bf16, name=f"ngb1b{b}", bufs=1)
                for b in range(BPC)]
        ngb2 = [spool.tile([128, MT], bf16, name=f"ngb2b{b}", bufs=1)
                for b in range(BPC)]
        clsb = [spool.tile([128, 1], f32, name=f"clsbb{b}", bufs=1)
                for b in range(BPC)]
        pcbs1 = [spool.tile([128, 2 * MT], f32, name=f"pcbs1b{b}", bufs=1)
                 for b in range(BPC)]
        pcbs2 = [spool.tile([128, 2 * MT], f32, name=f"pcbs2b{b}", bufs=1)
                 for b in range(BPC)]
        sv_t = spool.tile([128, 8], f32, name="sv_t", bufs=1)
        sv_c = [spool.tile([128, 8], f32, name=f"sv_c{b}", bufs=1)
                for b in range(BPC)]

        nc.vector.memset(textL, 0.0)
        nc.vector.memset(warm, 0.0)

        # ------------------------------------------------------------------
        # emission helpers
        # ------------------------------------------------------------------
        _l1_psum = {}
        _l1_gp0_tiles = {}

        def _l1ps(b, nsb):
            key = (b, nsb)
            if key not in _l1_psum:
                _l1_psum[key] = [
                    pa.tile([128, 512], f32, name=f"pa{m}", tag="pa") for m in range(MT)
                ]
            return _l1_psum[key]

        def l1_superblock(b, nsb, gp_range):
            """Layer1 MMs for one superblock's fetch-group range. The m2/m3
            accumulators defer their gp0 contribution to the end so their
            PSUM slots (reused from the previous superblock) have time to
            drain before their first write."""
            ns0 = nsb * 512
            for gp in gp_range:
                ftp = fpool.tile([128, 2, 2, 512], f8, name="ftp", tag="ftp")
                nc.sync.dma_start(
                    out=ftp, in_=feat[b, :, 2 * gp : 2 * gp + 2, :, ns0 : ns0 + 512]
                )
                for q in range(2):
                    g = 2 * gp + q
                    for m in range(MT):
                        if gp == 0 and m >= 2:
                            continue  # deferred below
                        nc.tensor.matmul(
                            _l1ps(b, nsb)[m],
                            lhsT=w1t_sb[:, g, :, m * 128 : (m + 1) * 128],
                            rhs=ftp[:, q, :, :],
                            start=(g == 0) if m < 2 else (g == 2),
                            stop=(g == GP - 1) if m < 2 else False,
                            perf_mode=PM.DoubleRow,
                        )
                if gp == 0:
                    _l1_gp0_tiles[(b, nsb)] = ftp

        def l1_finish(b, nsb):
            """Deferred gp0 MMs for m2/m3 (their stop), then drains."""
            ns0 = nsb * 512
            ftp = _l1_gp0_tiles.pop((b, nsb))
            ps = _l1_psum.pop((b, nsb))
            for q in range(2):
                for m in (2, 3):
                    nc.tensor.matmul(
                        ps[m],
                        lhsT=w1t_sb[:, q, :, m * 128 : (m + 1) * 128],
                        rhs=ftp[:, q, :, :],
                        start=False,
                        stop=(q == 1),
                        perf_mode=PM.DoubleRow,
                    )
            for m in range(MT):
                xs = x1p[b][:, m, ns0 : ns0 + 512]
                # x1p = ps/16 + 4*b1  (= 4 * x1_true); S1 accumulated on ACT
                nc.scalar.activation(
                    xs, ps[m], AF.Identity, bias=bcol(0, m), scale=S1A / SW,
                    accum_out=scq1[b][:, m, nsb, 0:1],
                )
                sqs = scpool.tile([128, 512], bf16, name="sqs", tag="sqs")
                nc.vector.scalar_tensor_tensor(
                    sqs, xs, 1.0, xs, op0=AL.bypass, op1=AL.mult,
                    accum_out=scq1[b][:, m, nsb, 1:2],
                )

        def text_mms(b):
            r0 = ROW[b]
            for nq in range(4):
                pstx = pb.tile([128, 512], f32, name="pstx", tag="pb")
                for k in range(DT):
                    nc.tensor.matmul(
                        pstx[:C],
                        lhsT=tpt_sb[:, k, :],
                        rhs=imgT[b][:, k, nq * 512 : (nq + 1) * 512],
                        start=(k == 0),
                        stop=(k == DT - 1),
                    )
                nc.vector.tensor_copy(textL[r0 : r0 + C, nq * 512 : (nq + 1) * 512],
                                      pstx[:C])

        def gn1_stats(b):
            """GN1 stats from staged (sum, sumsq) accumulators (scale S1A)."""
            psg = pb.tile([128, 512], f32, name=f"psg1b{b}", tag="pb")
            for m in range(MT):
                nc.tensor.matmul(
                    psg[:GROUPS, 0 : 2 * NSB],
                    lhsT=indi_sb[:, m, :],
                    rhs=scq1[b][:, m].rearrange("p a b -> p (a b)"),
                    start=(m == 0),
                    stop=(m == MT - 1),
                )
            grp = spool.tile([128, 4], f32, name=f"grp1b{b}", bufs=1)
            nc.vector.tensor_reduce(
                grp[:GROUPS, 0:2],
                psg[:GROUPS, 0 : 2 * NSB].rearrange("p (j s) -> p s j", j=NSB),
                AX.X, AL.add,
            )
            _gn_chain(grp, pcbs1[b], S1A, f"gn1b{b}")

        def gn2_stats(b):
            """GN2 stats from per-chunk bn_stats via bn_aggr (true scale)."""
            bna = spool.tile([128, MT, 2], f32, name=f"bna{b}", bufs=1)
            for m in range(MT):
                nc.vector.bn_aggr(bna[:, m, :],
                                  bnq2[b][:, m].rearrange("p a b -> p (a b)"))
                # col1 := var + mean^2  (per-channel mean square)
                nc.vector.scalar_tensor_tensor(
                    bna[:, m, 1:2], bna[:, m, 0:1], bna[:, m, 0:1],
                    bna[:, m, 1:2], op0=AL.mult, op1=AL.add,
                )
            psg = pb.tile([128, 512], f32, name=f"psg2b{b}", tag="pb")
            for m in range(MT):
                nc.tensor.matmul(
                    psg[:GROUPS, 0:2],
                    lhsT=indi2_sb[:, m, :],
                    rhs=bna[:, m, :],
                    start=(m == 0),
                    stop=(m == MT - 1),
                )
            grp = spool.tile([128, 4], f32, name=f"grp2b{b}", bufs=1)
            nc.vector.tensor_copy(grp[:GROUPS, 0:2], psg[:GROUPS, 0:2])
            _gn_chain(grp, pcbs2[b], 1.0, f"gn2b{b}")

        def _gn_chain(grp, pcbs, var_scale, lname):
            # grp cols: 0 = mu' (= s*mu), 1 = msq' (= s^2*msq)
            # mu'^2 - msq' = -s^2*var ; std = sqrt(-(x)/s^2 + eps)
            nc.vector.scalar_tensor_tensor(
                grp[:GROUPS, 2:3], grp[:GROUPS, 0:1], grp[:GROUPS, 0:1],
                grp[:GROUPS, 1:2], op0=AL.mult, op1=AL.subtract,
            )
            nc.scalar.activation(
                grp[:GROUPS, 2:3], grp[:GROUPS, 2:3], AF.Sqrt,
                bias=eps_sb[:GROUPS], scale=-1.0 / (var_scale * var_scale),
            )
            nc.vector.reciprocal(grp[:GROUPS, 2:3], grp[:GROUPS, 2:3])  # true rs
            nc.vector.tensor_tensor(
                grp[:GROUPS, 3:4], grp[:GROUPS, 2:3], grp[:GROUPS, 0:1], AL.mult
            )  # rs * mu'
            pcb = pb.tile([128, 512], f32, name=f"pcb_{lname}", tag="pb")
            for m in range(MT):
                nc.tensor.matmul(
                    pcb[:, 2 * m : 2 * m + 2],
                    lhsT=indj_sb[:GROUPS, m, :],
                    rhs=grp[:GROUPS, 2:4],
                    start=True,
                    stop=True,
                )
            nc.vector.tensor_copy(pcbs, pcb[:, 0 : 2 * MT])

        def fold1(b):
            # w2ts = W2^T * rs * (16*gamma1): with x1p = 4*x1 and SW=64 this
            # makes ps2 = 64 * W2 @ (gamma1*rs*x1)
            for k in range(MT):
                nc.vector.tensor_scalar(
                    w2ts[b][:, k, :], w2t_sb[:, k, :],
                    pcbs1[b][:, 2 * k : 2 * k + 1], bcol(4, k),
                    op0=AL.mult, op1=AL.mult,
                )
                # ngb1 = beta1 - gamma1*rs*mu   (gamma1n = -gamma1/4 hosted)
                nc.vector.tensor_scalar(
                    ngb1[b][:, k : k + 1], pcbs1[b][:, 2 * k + 1 : 2 * k + 2],
                    bcol(8, k), bcol(12, k), op0=AL.mult, op1=AL.add,
                )

        def psb_mm(b):
            psb = pb.tile([128, 512], f32, name=f"psb{b}", tag="pb")
            for m in range(MT):
                for k in range(MT):
                    nc.tensor.matmul(
                        psb[:, m : m + 1],
                        lhsT=w2t_sb[:, k, m * 128 : (m + 1) * 128],
                        rhs=ngb1[b][:, k : k + 1],
                        start=(k == 0),
                        stop=(k == MT - 1),
                    )
            # bias2 = b2 + W2 @ (beta1 - gamma1*rs*mu)
            nc.vector.tensor_tensor(bias2[b], bp_sb[:, 16:20], psb[:, 0:MT], AL.add)

        def l2_chunk(b, m, nsb):
            ns0 = nsb * 512
            ps2 = pb.tile([128, 512], f32, name="ps2", tag="pb")
            for q in range(2):
                nc.tensor.matmul(
                    ps2,
                    lhsT=w2ts[b][:, 2 * q : 2 * q + 2, m * 128 : (m + 1) * 128],
                    rhs=x1p[b][:, 2 * q : 2 * q + 2, ns0 : ns0 + 512],
                    start=(q == 0),
                    stop=(q == 1),
                    perf_mode=PM.DoubleRow,
                )
            xs = x2p[b][:, m, ns0 : ns0 + 512]
            nc.scalar.activation(
                xs, ps2, AF.Identity, bias=bias2[b][:, m : m + 1], scale=1.0 / SW,
            )
            nc.vector.bn_stats(bnq2[b][:, m, nsb, :], xs)

        def fold2(b):
            for k in range(DT):
                nc.vector.tensor_scalar(
                    wcts[b][:, k, :], wct_sb[:, k, :],
                    pcbs2[b][:, 2 * k : 2 * k + 1], bcol(20, k),
                    op0=AL.mult, op1=AL.mult,
                )
                nc.vector.tensor_scalar(
                    ngb2[b][:, k : k + 1], pcbs2[b][:, 2 * k + 1 : 2 * k + 2],
                    bcol(24, k), bcol(28, k), op0=AL.mult, op1=AL.add,
                )

        def pscb_mm(b):
            pscb = pb.tile([128, 512], f32, name=f"pscb{b}", tag="pb")
            for k in range(DT):
                nc.tensor.matmul(
                    pscb[:C, 0:1],
                    lhsT=wct_sb[:, k, :],
                    rhs=ngb2[b][:, k : k + 1],
                    start=(k == 0),
                    stop=(k == DT - 1),
                )
            nc.vector.tensor_tensor(clsb[b][:C], bc_sb[:C], pscb[:C, 0:1], AL.add)

        def cls_mms(b, nqs):
            r0 = ROW[b]
            for nq in nqs:
                psc = pb.tile([128, 512], f32, name="psc", tag="pb")
                for k in range(MT):
                    nc.tensor.matmul(
                        psc[:C],
                        lhsT=wcts[b][:, k, :],
                        rhs=x2p[b][:, k, nq * 512 : (nq + 1) * 512],
                        start=(k == 0),
                        stop=(k == MT - 1),
                    )
                nc.scalar.activation(
                    clsL[b][r0 : r0 + C, nq * 512 : (nq + 1) * 512],
                    psc[:C], AF.Sigmoid, bias=clsb[b][:C],
                )

        def transpose(b, k):
            nc.sync.dma_start_transpose(
                out=imgT[b][:, k, :], in_=img[b, :, k * 128 : (k + 1) * 128]
            )

        # ------------------------------------------------------------------
        # schedule
        # ------------------------------------------------------------------
        l1_superblock(0, 0, range(GPP)); l1_finish(0, 0)
        transpose(0, 0)
        l1_superblock(0, 1, range(GPP)); l1_finish(0, 1)
        transpose(0, 1)
        l1_superblock(0, 2, range(GPP)); l1_finish(0, 2)
        transpose(0, 2)
        l1_superblock(0, 3, range(GPP)); l1_finish(0, 3)
        transpose(0, 3)
        text_mms(0)

        l1_superblock(1, 0, range(GPP)); l1_finish(1, 0)
        transpose(1, 0)
        gn1_stats(0)
        l1_superblock(1, 1, range(2))
        fold1(0)
        l1_superblock(1, 1, range(2, GPP)); l1_finish(1, 1)
        transpose(1, 1)
        psb_mm(0)
        l1_superblock(1, 2, range(2))
        l2_chunk(0, 0, 0); l2_chunk(0, 1, 0)
        l1_superblock(1, 2, range(2, GPP)); l1_finish(1, 2)
        transpose(1, 2)
        l2_chunk(0, 2, 0); l2_chunk(0, 3, 0)
        l1_superblock(1, 3, range(2))
        l2_chunk(0, 0, 1); l2_chunk(0, 1, 1)
        l1_superblock(1, 3, range(2, GPP)); l1_finish(1, 3)
        transpose(1, 3)
        l2_chunk(0, 2, 1); l2_chunk(0, 3, 1)
        text_mms(1)
        tsearch = _search(nc, sv_t, junkT, junkAT, textL,
                          kp_sb[:, 0:1], kp_sb[:, 1:2], kp_sb[:, 2:3],
                          scout[:, 0:1], n_iters=T_ITERS, sample=T_SAMPLE,
                          unit_range=False)
        next(tsearch)  # init + iters 0-2
        l2_chunk(0, 0, 2); l2_chunk(0, 1, 2)
        gn1_stats(1)
        l2_chunk(0, 2, 2); l2_chunk(0, 3, 2)
        next(tsearch)  # iters 3-5
        l2_chunk(0, 0, 3); l2_chunk(0, 1, 3)
        fold1(1)
        l2_chunk(0, 2, 3); l2_chunk(0, 3, 3)
        psb_mm(1)
        next(tsearch)  # iters 6-7
        gn2_stats(0)
        fold2(0)
        pscb_mm(0)
        next(tsearch)  # final relu-sum + combine
        cls_mms(0, [0, 1])
        for nsb in range(2):
            for m in range(MT):
                l2_chunk(1, m, nsb)
        cls_mms(0, [2])
        for nsb in range(2, NSB):
            for m in range(MT):
                l2_chunk(1, m, nsb)
        cls_mms(0, [3])
        # pre-warm the Sqrt table after b0's sigmoids evicted it (data dep
        # on b0's last sigmoid chunk pins the ordering)
        nc.scalar.activation(warm, clsL[0][0:1, 2047:2048], AF.Sqrt)
        gn2_stats(1)
        csearch0 = _search(nc, sv_c[0], junkC[0], junkAC[0], clsL[0],
                           kp_sb[:, 3:4], kp_sb[:, 4:5], kp_sb[:, 5:6],
                           scout[:, 1:2], n_iters=C_ITERS, sample=C_SAMPLE,
                           unit_range=True, rows=slice(ROW[0], ROW[0] + C))
        next(csearch0)
        fold2(1)
        # pre-warm the Sigmoid table after GN2(b1)'s Sqrt (dep on pcbs2[1])
        nc.scalar.activation(warm, pcbs2[1][0:1, 0:1], AF.Sigmoid)
        pscb_mm(1)
        cls_mms(1, [0])
        csearch1 = _search(nc, sv_c[1], junkC[1], junkAC[1], clsL[1],
                           kp_sb[:, 3:4], kp_sb[:, 4:5], kp_sb[:, 5:6],
                           scout[:, 1:2], n_iters=C_ITERS, sample=C_SAMPLE,
                           unit_range=True, rows=slice(ROW[1], ROW[1] + C))
        next(csearch1)  # init + iters 0-2 (sample needs only cols 0:512 = nq0)
        cls_mms(1, [1, 2, 3])
        next(csearch0)
        next(csearch1)  # iters 3-5
        for _ in csearch0:
            pass
        for _ in csearch1:
            pass
        nc.sync.dma_start(out=scores.ap(), in_=scout)
        if DEBUG:
            nc.sync.dma_start(out=io["dbg_text"].ap(), in_=textL)
            nc.sync.dma_start(out=io["dbg_cls"].ap(), in_=clsL[0])
            nc.sync.dma_start(out=io["dbg_cls1"].ap(), in_=clsL[1])


_PROG = None


def _build_program():
    global _PROG
    if _PROG is not None:
        return _PROG
    nc = bacc.Bacc("TRN2", target_bir_lowering=False, debug=False)
    io = {}
    io["feat"] = nc.declare_dram_parameter("feat", [BPC, 128, GP, 2, T], f8, isOutput=False).ap()
    io["img"] = nc.declare_dram_parameter("img", [BPC, TIMG, D], bf16, isOutput=False).ap()
    io["w1t"] = nc.declare_dram_parameter("w1t", [128, GP, 2, O], f8, isOutput=False).ap()
    io["w2t"] = nc.declare_dram_parameter("w2t", [128, MT, O], bf16, isOutput=False).ap()
    io["wct"] = nc.declare_dram_parameter("wct", [128, DT, C], bf16, isOutput=False).ap()
    io["tpt"] = nc.declare_dram_parameter("tpt", [128, DT, C], bf16, isOutput=False).ap()
    io["bias_pack"] = nc.declare_dram_parameter("bias_pack", [128, 32], f32, isOutput=False).ap()
    io["bc_pad"] = nc.declare_dram_parameter("bc_pad", [128, 1], f32, isOutput=False).ap()
    io["ind_i"] = nc.declare_dram_parameter("ind_i", [128, MT, GROUPS], f32, isOutput=False).ap()
    io["ind_i2"] = nc.declare_dram_parameter("ind_i2", [128, MT, GROUPS], f32, isOutput=False).ap()
    io["ind_j"] = nc.declare_dram_parameter("ind_j", [GROUPS, MT, 128], f32, isOutput=False).ap()
    io["kpack"] = nc.declare_dram_parameter("kpack", [128, 6], f32, isOutput=False).ap()
    io["scores"] = nc.declare_dram_parameter("scores", [128, 2], f32, isOutput=True)
    if DEBUG:
        io["dbg_text"] = nc.declare_dram_parameter("dbg_text", [128, T], bf16, isOutput=True)
        io["dbg_cls"] = nc.declare_dram_parameter("dbg_cls", [128, T], bf16, isOutput=True)
        io["dbg_cls1"] = nc.declare_dram_parameter("dbg_cls1", [128, T], bf16, isOutput=True)
    with tile.TileContext(nc) as tc:
        _body(tc, io)
    nc.compile()
    _PROG = nc
    return nc


def build_in_maps(input_features, masks, text_proto, img_feats, img_masks,
                  W1, b1, g1, beta1, W2, b2, g2, beta2, Wc, bc):
    """Host-side prep: shard activations per core, pack params (replicated)."""
    asf = lambda a: np.ascontiguousarray(a, dtype=np.float32)
    asb = lambda a: np.ascontiguousarray(np.asarray(a, np.float32).astype(ml_dtypes.bfloat16))
    as8 = lambda a: np.ascontiguousarray(np.asarray(a, np.float32).astype(ml_dtypes.float8_e4m3))

    # feat: [B, FD, T] -> [B, 128(p), GP(g), 2(j), T] with FD = 256g+128j+p
    featdr = as8(np.asarray(input_features, np.float32)
                 .reshape(B, GP, 2, 128, T).transpose(0, 3, 1, 2, 4))
    imgb = asb(img_feats)

    # w1t: [128(p), GP, 2, O] with w1t[p,g,j,o] = 64*W1[o, 256g+128j+p]
    w1s = np.asarray(W1, np.float32).T * SW          # [FD, O]
    w1dr = as8(w1s.reshape(GP, 2, 128, O).transpose(2, 0, 1, 3))
    w2tb = asb(np.asarray(W2, np.float32).T.reshape(MT, 128, O).transpose(1, 0, 2))
    wctb = asb(np.asarray(Wc, np.float32).T.reshape(DT, 128, C).transpose(1, 0, 2))
    tptb = asb(np.asarray(text_proto, np.float32)[0].T.reshape(DT, 128, C).transpose(1, 0, 2))

    bias_pack = np.zeros((128, 32), np.float32)
    cols = [
        np.asarray(b1, np.float32) * S1A,       # 0-3: 4*b1 (drain bias)
        np.asarray(g1, np.float32) * 16.0,      # 4-7: 16*gamma1 (w2ts fold)
        np.asarray(g1, np.float32) * (-0.25),   # 8-11: -gamma1/4 (ngb1)
        np.asarray(beta1, np.float32),          # 12-15
        np.asarray(b2, np.float32),             # 16-19
        np.asarray(g2, np.float32),             # 20-23
        np.asarray(g2, np.float32) * (-1.0),    # 24-27
        np.asarray(beta2, np.float32),          # 28-31
    ]
    for i, v in enumerate(cols):
        bias_pack[:, 4 * i : 4 * i + 4] = v.reshape(MT, 128).T
    bc_pad = np.zeros((128, 1), np.float32)
    bc_pad[:C, 0] = np.asarray(bc, np.float32)

    p = np.arange(128)
    ind_i = np.zeros((128, MT, GROUPS), np.float32)
    ind_i2 = np.zeros((128, MT, GROUPS), np.float32)
    ind_j = np.zeros((GROUPS, MT, 128), np.float32)
    for m in range(MT):
        ind_i[p, m, m * 8 + p // 16] = 1.0 / GN_N
        ind_i2[p, m, m * 8 + p // 16] = 1.0 / (O // GROUPS)
        ind_j[m * 8 + p // 16, m, p] = 1.0

    text_len = np.asarray(img_masks, np.float32).sum(-1).astype(np.int64)
    cls_len = np.asarray(masks, np.float32).sum((-2, -1)).astype(np.int64)
    k_text = np.maximum(1, text_len // R_ACT)
    k_cls = np.maximum(1, cls_len // R_ACT)

    in_maps = []
    for c in range(NCORES):
        bb = (BPC * c, BPC * c + 1)
        kpack = np.zeros((128, 6), np.float32)
        kpack[:, 0] = round(256.0 * T_SAMPLE / T)
        kpack[:, 1] = 256.0
        kpack[:, 2] = 1.0 / 256.0
        kpack[:, 3] = round(256.0 * C_SAMPLE / T)
        kpack[:, 4] = 256.0
        kpack[:, 5] = 1.0 / 256.0
        for i, b_ in enumerate(bb):
            r = ROW[i]
            kpack[r : r + C, 0] = np.round(k_text[b_] * T_SAMPLE / T)
            kpack[r : r + C, 1] = k_text[b_]
            kpack[r : r + C, 2] = 1.0 / k_text[b_]
            kpack[r : r + C, 3] = np.round(k_cls[b_] * C_SAMPLE / T)
            kpack[r : r + C, 4] = k_cls[b_]
            kpack[r : r + C, 5] = 1.0 / k_cls[b_]
        in_maps.append({
            "feat": featdr[bb[0] : bb[1] + 1],
            "img": imgb[bb[0] : bb[1] + 1],
            "w1t": w1dr, "w2t": w2tb, "wct": wctb, "tpt": tptb,
            "bias_pack": bias_pack, "bc_pad": bc_pad,
            "ind_i": ind_i, "ind_i2": ind_i2, "ind_j": ind_j,
            "kpack": kpack,
        })
    return in_maps


def assemble_output(results):
    out = np.zeros((2, B, C), np.float32)
    for c in range(NCORES):
        s = np.asarray(results[c]["scores"]).reshape(128, 2)
        for i in range(BPC):
            r = ROW[i]
            out[0, BPC * c + i] = s[r : r + C, 0]
            out[1, BPC * c + i] = s[r : r + C, 1]
    return out


def _numpy_reference(input_features, masks, text_proto, img_feats, img_masks,
                     W1, b1, g1, beta1, W2, b2, g2, beta2, Wc, bc):
    """Exact numpy fallback, used only if masks are not all-ones."""
    def gn(x, gamma, beta):
        b_, c_, t_ = x.shape
        xr = x.reshape(b_, GROUPS, c_ // GROUPS, t_)
        mu = xr.mean(axis=(2, 3), keepdims=True)
        var = xr.var(axis=(2, 3), keepdims=True)
        xn = ((xr - mu) / np.sqrt(var + EPS)).reshape(b_, c_, t_)
        return xn * gamma[None, :, None] + beta[None, :, None]

    def topk_mean(logits, valid_len):
        vals = -np.sort(-logits, axis=1)
        csum = np.cumsum(vals, axis=1)
        k = np.maximum(1, valid_len // R_ACT).astype(np.int64)
        sel = np.take_along_axis(csum, (k - 1)[:, None, None].repeat(C, 2), axis=1)[:, 0, :]
        return sel / k[:, None]

    x = np.einsum("of,bft->bot", W1, input_features) + b1[None, :, None]
    x = gn(x, g1, beta1) * masks
    x = np.einsum("oc,bct->bot", W2, x) + b2[None, :, None]
    x = gn(x, g2, beta2) * masks
    fe = x.transpose(0, 2, 1)
    cls_logits = 1.0 / (1.0 + np.exp(-(np.einsum("bto,co->btc", fe, Wc) + bc)))
    tp = text_proto[0].T
    text_logits = np.einsum("btd,dc->btc", img_feats, tp)
    text_len = img_masks.sum(-1).astype(np.int64)
    cls_len = masks.sum((-2, -1)).astype(np.int64)
    return np.stack([
        topk_mean(text_logits, text_len),
        topk_mean(cls_logits, cls_len),
    ]).astype(np.float32)


def kernel(**inputs):
    inputs = {k: np.asarray(v) for k, v in inputs.items()}
    masks = inputs["masks"]
    img_masks = inputs["img_masks"]
    if not (np.all(masks == 1.0) and np.all(img_masks == 1.0)):
        # masked GN/logits differ when masks are non-trivial; use exact host path
        return _numpy_reference(**{k: v.astype(np.float32) for k, v in inputs.items()})
    nc = _build_program()
    in_maps = build_in_maps(**inputs)
    res = run_bass_kernel_spmd(nc, in_maps, list(range(NCORES)))
    return assemble_output(res.results)


if __name__ == "__main__":
    import jax
    import reference
    with jax.default_device(jax.devices("cpu")[0]):
        inp = {k: np.asarray(v) for k, v in reference.setup_inputs().items()}
        exp = np.asarray(reference.reference(**inp))
    act = kernel(**inp)
    err = np.abs(act - exp).max() / (np.abs(exp).max() + 1e-12)
    print("max abs err:", np.abs(act - exp).max(), "rel:", err)
